# revision 1
# baseline (speedup 1.0000x reference)
"""BitPackedLinear Trainium2 kernel (8-core SPMD, token-sharded).

y = x @ W.T + bias, W = unpack_bits(packed_weight) in {-1,+1}, shapes:
  x [2, 2048, 4096] f32, packed_weight [4096, 512] u8, bias [4096] f32.

Sharding: data-parallel over tokens (4096 tokens -> 512/core). Each core
computes y_c = x_c @ W.T + bias for its token shard against the full
weight; the host just concatenates shards.

Device algorithm per core (all matmuls bf16 at 1 cyc/row):
  - Contraction (i) is tiled bit-sliced: i-tile (kt, b) = {8*(128*kt+k')+b},
    which makes every W^T [i,o] tile a single shift+mask op from transposed
    packed bytes (no cross-partition fanout).
  - byteT[k',kt,o] = pw[o,128*kt+k'] via PE pass-through transposes,
    built just-in-time per o-slab.
  - Unpack writes the bf16 BIT PATTERN of {0, 2.0} with pure bitvec ops:
    (u16(byte) << (14-b)) & 0x4000, then the tile is bitcast to bf16.
    (walrus forbids dtype casts on bitvec tensor_scalar ops.)
  - x is cast f32->bf16 in-flight by SWDGE DMA; xT tiles via PE transpose
    (bf16, 1 cyc/row) in i-tile order so the matmul stream chases them.
  - main matmuls: psum[t,o] += xT_it.T @ W2_it  (= 2*(x@B.T), y-natural)
  - bias via a rank-1 f32r matmul (K=1): psum += ones.T @ bias_row.
  - rowsum correction: s_col[t] = sum_i x_bf[t,i] via DVE reduces of the
    bf16 x chunks (exactly consistent with the bf16 matmul inputs);
    epilogue fuses y = psum - s_col into the PSUM->SBUF copy.
"""
import sys

sys.path.insert(0, "/opt/trn_rl_repo")
from contextlib import ExitStack

import numpy as np

import concourse.tile as tile
from concourse import bacc, mybir
from concourse.bass import ts
from concourse.bass_utils import run_bass_kernel_spmd
from concourse.masks import make_identity

F32 = mybir.dt.float32
F32R = mybir.dt.float32r
BF16 = mybir.dt.bfloat16
U8 = mybir.dt.uint8
U16 = mybir.dt.uint16
P = 128

N_CORES = 8
B_DIM, S_DIM, I_DIM, O_DIM = 2, 2048, 4096, 4096
T_FULL = B_DIM * S_DIM          # 4096 tokens
T_SHARD = T_FULL // N_CORES     # 512 tokens per core
OUT_NAME = "y"
OUT_SHAPE = (T_SHARD, O_DIM)


def build(T=T_SHARD, I=I_DIM, O=O_DIM, O_SLAB=512, n_cores=N_CORES, n_reps=1,
          byte_mode="pe"):
    assert I % 1024 == 0 and T % P == 0 and O % P == 0 and O % O_SLAB == 0
    KT = I // 1024          # 128-byte groups along i
    IT = KT * 8             # bit-sliced i-tiles
    TT = T // P             # token tiles
    K = I // 8              # packed bytes per weight row
    NSLAB = O // O_SLAB
    OSL_T = O_SLAB // P
    SHB, MASK = 14, 0x4000  # u16 bf16-pattern unpack constants

    nc = bacc.Bacc("TRN2", target_bir_lowering=False, debug=False,
                   num_devices=n_cores)
    x_d = nc.dram_tensor("x", [T, I], F32, kind="ExternalInput").ap()
    pw_d = nc.dram_tensor("pw", [O, K], U8, kind="ExternalInput").ap()
    bias_d = nc.dram_tensor("bias", [O], F32, kind="ExternalInput").ap()
    y_d = nc.dram_tensor(OUT_NAME, [T, O], F32, kind="ExternalOutput").ap()

    with tile.TileContext(nc) as tc:
        with ExitStack() as ctx:
            const = ctx.enter_context(tc.tile_pool(name="const", bufs=1))
            persist = ctx.enter_context(tc.tile_pool(name="persist", bufs=1))

            ident_bf = const.tile([P, P], BF16)
            make_identity(nc, ident_bf[:])
            ones_r = const.tile([1, P], F32R)
            bias_r = const.tile([1, O], F32R)
            stage = ctx.enter_context(tc.tile_pool(name="stage", bufs=1))

            def emit_bias_stage():
                ones_f32 = stage.tile([1, P], F32)
                nc.vector.memset(ones_f32[:], 1.0)
                nc.vector.tensor_copy(out=ones_r[:], in_=ones_f32[:])
                bias_f32 = stage.tile([1, O], F32)
                nc.sync.dma_start(
                    bias_f32[:], bias_d.rearrange("(b o) -> b o", b=1)
                )
                nc.vector.tensor_copy(out=bias_r[:], in_=bias_f32[:])

            byteT = persist.tile([P, KT, O], U16)
            xT = persist.tile([P, IT, T], BF16)
            pw_ap = pw_d.rearrange("(ot p) k -> p ot k", p=P)
            pw16_d = nc.dram_tensor("pw16", [O, K], U16).ap()

            pk_pool = ctx.enter_context(tc.tile_pool(name="pk", bufs=2))
            pkbf_pool = ctx.enter_context(tc.tile_pool(name="pkbf", bufs=2))
            ps_tr = ctx.enter_context(
                tc.tile_pool(name="ps_tr", bufs=3, space="PSUM")
            )
            xnat_pool = ctx.enter_context(
                tc.tile_pool(name="xnat", bufs=max(2 * TT, KT * TT - 4))
            )
            x32_pool = ctx.enter_context(tc.tile_pool(name="x32", bufs=2))
            scol_pool = ctx.enter_context(tc.tile_pool(name="scol", bufs=2))
            wt_pool = ctx.enter_context(tc.tile_pool(name="wt", bufs=2))
            y_pool = ctx.enter_context(tc.tile_pool(name="ysb", bufs=3))
            ps_mm = ctx.enter_context(
                tc.tile_pool(name="ps_mm", bufs=4, space="PSUM")
            )
            ps_b_pool = ctx.enter_context(
                tc.tile_pool(name="ps_b", bufs=1, space="PSUM")
            )

            def byte_slab(sl):
                """Fill byteT[:, :, sl*O_SLAB:(sl+1)*O_SLAB] from pw (JIT)."""
                pk = pk_pool.tile([P, OSL_T, K], U8)
                nc.sync.dma_start(pk[:], pw_ap[:, ts(sl, OSL_T), :])
                for otl in range(OSL_T):
                    ot = sl * OSL_T + otl
                    pkbf = pkbf_pool.tile([P, K], BF16)
                    nc.any.tensor_copy(out=pkbf[:], in_=pk[:, otl, :])
                    for kt in range(KT):
                        ps = ps_tr.tile([P, P], BF16, tag="tr_ps")
                        nc.tensor.transpose(ps[:], pkbf[:, ts(kt, P)], ident_bf[:])
                        nc.any.tensor_copy(out=byteT[:, kt, ts(ot, P)], in_=ps[:])

            for _rep in range(n_reps):
                if byte_mode == "dmat":
                    # bytes: u8->u16 cast bounce through DRAM, then one
                    # XBAR transpose-DMA per kt fills byteT[:, kt, :] whole
                    nc.gpsimd.dma_start(out=pw16_d[:], in_=pw_d[:])
                    for kt in range(KT):
                        nc.sync.dma_start_transpose(
                            byteT[:, kt, :], pw16_d[:, ts(kt, P)]
                        )
                else:
                    # slab 0 first: DVE/ACT work exists while x DMAs land
                    # (hybrid mode: slabs 1+ come via XBAR, emitted later)
                    byte_slab(0)

                # x chunks, kt-major. kt=0 goes via fast HWDGE as f32 +
                # a DVE cast (DVE is idle this early; SWDGE descriptor-gen
                # on the Q7 makes the first cast-DMA land ~13us in, which
                # stalls the PE). kt>=1 use SWDGE cast-DMA f32->bf16.
                xns = {}
                for kt in range(KT):
                    for tt in range(TT):
                        src_ap = x_d[ts(tt, P), ts(kt, 1024)].rearrange(
                            "p (k b) -> p k b", b=8
                        )
                        xn = xnat_pool.tile([P, P, 8], BF16, tag="xn16")
                        if kt == 0:
                            x32 = x32_pool.tile([P, P, 8], F32)
                            nc.sync.dma_start(x32[:], src_ap)
                            nc.vector.tensor_copy(out=xn[:], in_=x32[:])
                        else:
                            nc.gpsimd.dma_start(xn[:], src_ap)
                        xns[kt, tt] = xn

                if byte_mode == "hybrid":
                    # slabs 1..7 bytes via XBAR transpose-DMAs; cast-DMA is
                    # emitted after the x chunks so it loses the Q7 race.
                    # Each dest is one contiguous 1KB run per partition.
                    nc.gpsimd.dma_start(
                        out=pw16_d[O_SLAB:, :], in_=pw_d[O_SLAB:, :]
                    )
                    for sl in range(1, NSLAB):
                        for kt in range(KT):
                            nc.sync.dma_start_transpose(
                                byteT[:, kt, ts(sl, O_SLAB)],
                                pw16_d[ts(sl, O_SLAB), ts(kt, P)],
                            )

                # xT via PE transposes, i-tile-major so matmuls can chase
                for kt in range(KT):
                    for b in range(8):
                        it = kt * 8 + b
                        for tt in range(TT):
                            ps = ps_tr.tile([P, P], BF16, tag="tr_ps")
                            nc.tensor.transpose(
                                ps[:], xns[kt, tt][:, :, b], ident_bf[:]
                            )
                            nc.any.tensor_copy(out=xT[:, it, ts(tt, P)], in_=ps[:])

                if _rep == 0:
                    emit_bias_stage()

                # s_col[t] = sum_i x_bf[t, i] on DVE (consistent with MM
                # input); emitted after slab 0's unpack so it doesn't delay
                # the first matmuls
                s_col = scol_pool.tile([P, TT], F32)
                parts = scol_pool.tile([P, TT, KT], F32, tag="sparts")

                def emit_s_col():
                    for tt in range(TT):
                        for kt in range(KT):
                            nc.vector.tensor_reduce(
                                out=parts[:, tt, kt:kt + 1],
                                in_=xns[kt, tt][:],
                                op=mybir.AluOpType.add,
                                axis=mybir.AxisListType.XY,
                            )
                        nc.vector.tensor_reduce(
                            out=s_col[:, tt:tt + 1],
                            in_=parts[:, tt, :],
                            op=mybir.AluOpType.add,
                            axis=mybir.AxisListType.X,
                        )

                # main o-slab loop
                for sl in range(NSLAB):
                    if sl > 0 and byte_mode == "pe":
                        byte_slab(sl)
                    wt = wt_pool.tile([P, IT, O_SLAB], U16)
                    for kt in range(KT):
                        for b in range(8):
                            it = kt * 8 + b
                            nc.vector.tensor_scalar(
                                out=wt[:, it, :],
                                in0=byteT[:, kt, ts(sl, O_SLAB)],
                                scalar1=SHB - b, scalar2=MASK,
                                op0=mybir.AluOpType.logical_shift_left,
                                op1=mybir.AluOpType.bitwise_and,
                            )
                    if sl == 0:
                        emit_s_col()
                    ps_bias = ps_b_pool.tile([P, O_SLAB], F32)
                    nc.tensor.matmul(
                        ps_bias[:], ones_r[:], bias_r[:, ts(sl, O_SLAB)],
                        start=True, stop=True,
                    )
                    bbc = y_pool.tile([P, O_SLAB], F32, name="bbc", tag="y_sb")
                    nc.any.tensor_copy(out=bbc[:], in_=ps_bias[:])
                    for tsub in range(TT):
                        ps = ps_mm.tile([P, O_SLAB], F32)
                        for it in range(IT):
                            nc.tensor.matmul(
                                ps[:],
                                xT[:, it, ts(tsub, P)],
                                wt[:, it, :].bitcast(BF16),
                                start=(it == 0), stop=(it == IT - 1),
                            )
                        y_sb = y_pool.tile([P, O_SLAB], F32)
                        nc.vector.scalar_tensor_tensor(
                            out=y_sb[:], in0=ps[:],
                            scalar=s_col[:, tsub:tsub + 1],
                            in1=bbc[:],
                            op0=mybir.AluOpType.subtract,
                            op1=mybir.AluOpType.add,
                        )
                        nc.sync.dma_start(
                            y_d[ts(tsub, P), ts(sl, O_SLAB)], y_sb[:]
                        )

    nc.compile()
    return nc


_NC = None


def _get_nc():
    global _NC
    if _NC is None:
        _NC = build()
    return _NC


def run(x, packed_weight, bias, trace=False):
    x = np.ascontiguousarray(np.asarray(x, dtype=np.float32))
    pw = np.ascontiguousarray(np.asarray(packed_weight, dtype=np.uint8))
    bias = np.ascontiguousarray(np.asarray(bias, dtype=np.float32))
    assert x.shape == (B_DIM, S_DIM, I_DIM)
    assert pw.shape == (O_DIM, I_DIM // 8)
    assert bias.shape == (O_DIM,)

    nc = _get_nc()
    xs = x.reshape(T_FULL, I_DIM)
    in_maps = [
        {
            "x": np.ascontiguousarray(xs[c * T_SHARD:(c + 1) * T_SHARD]),
            "pw": pw,
            "bias": bias,
        }
        for c in range(N_CORES)
    ]
    res = run_bass_kernel_spmd(nc, in_maps, list(range(N_CORES)), trace=trace)
    y = np.concatenate(
        [res.results[c][OUT_NAME] for c in range(N_CORES)], axis=0
    )
    return y.reshape(B_DIM, S_DIM, O_DIM), res


def kernel(x, packed_weight, bias):
    y, _ = run(x, packed_weight, bias, trace=False)
    return y



# revision 11
# speedup vs baseline: 1.3390x; 1.3390x over previous
"""BitPackedLinear Trainium2 kernel (8-core SPMD, token-sharded, fp8 DR).

y = x @ W.T + bias, W = unpack_bits(packed_weight) in {-1,+1}, shapes:
  x [2, 2048, 4096] f32, packed_weight [4096, 512] u8, bias [4096] f32.

Sharding: data-parallel over tokens (4096 tokens -> 512/core). Each core
computes y_c = x_c @ W.T + bias for its token shard against the full
weight; the host just concatenates shards.

Device algorithm per core (main GEMM in fp8e4 DoubleRow, 0.5 cyc/row --
2x the bf16 row rate, contracting two 128-i blocks per instruction):
  - x is split exactly as x = hi + lo with hi = fp8(bf16(x)),
    lo = fp8(bf16(x) - hi); W is unpacked to {0, 2.0} and the result
    corrected as y = (hi+lo)@(2B) - rowsum(hi+lo) + bias.  Total split
    error ~1.2e-3 rel (bf16-level).
  - x chunks arrive bf16 (SWDGE cast-DMA; kt=0 via HWDGE f32 + DVE cast
    to dodge the slow Q7 descriptor-gen start), bit-sliced layout.
  - PE transposes x per [128,128] bf16 tile into per-kt groups, then one
    ACT copy casts the group to the fp8 hi plane and one DVE subtract
    produces the fp8 lo plane: xT_hi/xT_lo [128 i-part, 32 it, 512 tok],
    tokens contiguous (the dual-fp8 Ldweights ISA check requires a
    unit-stride stationary m dim; the moving ifmap tolerates stride 2).
  - byteT[k',kt,o] = pw[o,128kt+k'] via u8->u16 cast-DMA bounce through
    DRAM + one XBAR transpose-DMA per kt (no PE/DVE involvement).
  - W unpack stays 1 op/tile: wt16 = (u16(byte) << (14-b)) & 0x4000
    puts fp8 {0, 2.0} in the HIGH byte; the DR matmul reads odd bytes
    ([p, 2, 512] pair-stride 1024, n-stride 2 -- validated on hw).
  - Main DR matmuls per (o-slab, token-tile), for j in 0..16, term in
    (hi, lo): psum += plane[:, 2j:2j+2, t].T (x) wt8[:, 2j:2j+2, :].
  - s_col = rowsum(hi+lo) computed EXACTLY by 32 DR matmuls against an
    all-ones stationary -> psum row -> SBUF f32r row.
  - bias + (-s_col) enter each psum via ONE rank-2 f32r matmul
    (k=0: s_col x -1, k=1: ones x bias) with start=True; epilogue is a
    plain psum->SBUF copy + DMA.
"""
import sys

sys.path.insert(0, "/opt/trn_rl_repo")
from contextlib import ExitStack

import numpy as np

import concourse.tile as tile
from concourse import bacc, mybir
from concourse.bass import ts
from concourse.bass_utils import run_bass_kernel_spmd
from concourse.masks import make_identity

F32 = mybir.dt.float32
F32R = mybir.dt.float32r
BF16 = mybir.dt.bfloat16
F8 = mybir.dt.float8e4
U8 = mybir.dt.uint8
U16 = mybir.dt.uint16
P = 128
DR = mybir.MatmulPerfMode.DoubleRow
ACT_COPY = mybir.ActivationFunctionType.Copy

N_CORES = 8
B_DIM, S_DIM, I_DIM, O_DIM = 2, 2048, 4096, 4096
T_FULL = B_DIM * S_DIM          # 4096 tokens
T_SHARD = T_FULL // N_CORES     # 512 tokens per core
OUT_NAME = "y"
OUT_SHAPE = (T_SHARD, O_DIM)


def build(T=T_SHARD, I=I_DIM, O=O_DIM, O_SLAB=512, n_cores=N_CORES,
          byte_mode="dmat"):
    assert I % 1024 == 0 and T % P == 0 and O % P == 0 and O % O_SLAB == 0
    KT = I // 1024          # 128-byte groups along i (4)
    IT = KT * 8             # bit-sliced i-tiles (32)
    NJ = IT // 2            # DR i-tile pairs (16)
    TT = T // P             # token tiles (4)
    K = I // 8              # packed bytes per weight row (512)
    NSLAB = O // O_SLAB
    OSL_T = O_SLAB // P
    SHB, MASK = 14, 0x4000  # unpack: fp8 {0,2.0} pattern in the HIGH byte

    nc = bacc.Bacc("TRN2", target_bir_lowering=False, debug=False,
                   num_devices=n_cores)
    x_d = nc.dram_tensor("x", [T, I], F32, kind="ExternalInput").ap()
    pw_d = nc.dram_tensor("pw", [O, K], U8, kind="ExternalInput").ap()
    bias_d = nc.dram_tensor("bias", [O], F32, kind="ExternalInput").ap()
    y_d = nc.dram_tensor(OUT_NAME, [T, O], F32, kind="ExternalOutput").ap()
    pw16_d = nc.dram_tensor("pw16", [O, K], U16).ap()

    with tile.TileContext(nc) as tc:
        with ExitStack() as ctx:
            const = ctx.enter_context(tc.tile_pool(name="const", bufs=1))
            persist = ctx.enter_context(tc.tile_pool(name="persist", bufs=1))
            stage = ctx.enter_context(tc.tile_pool(name="stage", bufs=1))

            ident_bf = const.tile([P, P], BF16)
            make_identity(nc, ident_bf[:])
            ones2 = const.tile([P, 2, P], F8)
            nc.vector.memset(ones2[:], 1.0)

            # rank-2 psum-init operands; engine-written rows sit at
            # partition 0 (engine APs must start on a quarter boundary),
            # partition-1 rows are DMA-filled.
            # ps = s2[0,t]*br2[0,o] + s2[1,t]*br2[1,o]
            #    = s_col[t]*(-1)    + 1*bias[o]
            s2 = const.tile([2, T], F32R)    # row0 = s_col, row1 = ones
            br2 = const.tile([2, O], F32R)   # row0 = -ones, row1 = bias

            byteT = persist.tile([P, KT, O], U16)
            xT_hi = persist.tile([P, IT, T], F8)
            xT_lo = persist.tile([P, IT, T], F8)

            x32_pool = ctx.enter_context(tc.tile_pool(name="x32", bufs=2))
            xn_pool = ctx.enter_context(tc.tile_pool(name="xnat", bufs=10))
            xtb_pool = ctx.enter_context(tc.tile_pool(name="xtb", bufs=2))
            pk_pool = ctx.enter_context(tc.tile_pool(name="pk", bufs=2))
            pkbf_pool = ctx.enter_context(tc.tile_pool(name="pkbf", bufs=2))
            wt_pool = ctx.enter_context(tc.tile_pool(name="wt", bufs=2))
            y_pool = ctx.enter_context(tc.tile_pool(name="ysb", bufs=3))
            ps_tr = ctx.enter_context(
                tc.tile_pool(name="ps_tr", bufs=3, space="PSUM")
            )
            ps_mm = ctx.enter_context(
                tc.tile_pool(name="ps_mm", bufs=4, space="PSUM")
            )
            ps_s_pool = ctx.enter_context(
                tc.tile_pool(name="ps_s", bufs=1, space="PSUM")
            )

            def emit_bias_stage():
                nc.sync.dma_start(
                    br2[1:2, :],
                    bias_d.bitcast(F32R).rearrange("(b o) -> b o", b=1),
                )
                cst = stage.tile([1, T], F32)
                nc.vector.memset(cst[:], -1.0)
                nc.vector.tensor_copy(
                    out=br2[0:1, :],
                    in_=cst[:, :1].broadcast_to([1, O]),
                )
                ones_row = stage.tile([1, T], F32R)
                nc.vector.memset(cst[:], 1.0)
                nc.vector.tensor_copy(out=ones_row[:], in_=cst[:])
                nc.sync.dma_start(s2[1:2, :], ones_row[:])

            def byte_slab(sl):
                """Fill byteT[:, :, sl*O_SLAB:(sl+1)*O_SLAB] via PE."""
                pw_ap = pw_d.rearrange("(ot p) k -> p ot k", p=P)
                pk = pk_pool.tile([P, OSL_T, K], U8)
                nc.sync.dma_start(pk[:], pw_ap[:, ts(sl, OSL_T), :])
                for otl in range(OSL_T):
                    ot = sl * OSL_T + otl
                    pkbf = pkbf_pool.tile([P, K], BF16)
                    nc.any.tensor_copy(out=pkbf[:], in_=pk[:, otl, :])
                    for kt in range(KT):
                        ps = ps_tr.tile([P, P], BF16, tag="tr_ps")
                        nc.tensor.transpose(ps[:], pkbf[:, ts(kt, P)], ident_bf[:])
                        nc.any.tensor_copy(out=byteT[:, kt, ts(ot, P)], in_=ps[:])

            # ---- byte path ----
            if byte_mode == "dmat":
                nc.gpsimd.dma_start(out=pw16_d[:], in_=pw_d[:])
                for kt in range(KT):
                    nc.sync.dma_start_transpose(
                        byteT[:, kt, :], pw16_d[:, ts(kt, P)]
                    )
            else:
                byte_slab(0)

            emit_bias_stage()

            # ---- x chunks, kt-major; kt=0 via HWDGE f32 + DVE cast ----
            xns = {}
            for kt in range(KT):
                for tt in range(TT):
                    src_ap = x_d[ts(tt, P), ts(kt, 1024)].rearrange(
                        "p (k b) -> p k b", b=8
                    )
                    xn = xn_pool.tile([P, P, 8], BF16, tag="xn16")
                    if kt == 0:
                        x32 = x32_pool.tile([P, P, 8], F32)
                        nc.sync.dma_start(x32[:], src_ap)
                        nc.vector.tensor_copy(out=xn[:], in_=x32[:])
                    else:
                        nc.gpsimd.dma_start(xn[:], src_ap)
                    xns[kt, tt] = xn

            # ---- transpose (bf16) per kt group, then split hi/lo fp8 ----
            for kt in range(KT):
                xtb = xtb_pool.tile([P, 8, T], BF16)
                for b in range(8):
                    for tt in range(TT):
                        ps = ps_tr.tile([P, P], BF16, tag="tr_ps")
                        nc.tensor.transpose(
                            ps[:], xns[kt, tt][:, :, b], ident_bf[:]
                        )
                        nc.any.tensor_copy(out=xtb[:, b, ts(tt, P)], in_=ps[:])
                hi = xT_hi[:, ts(kt, 8), :]
                lo = xT_lo[:, ts(kt, 8), :]
                nc.scalar.activation(out=hi, in_=xtb[:], func=ACT_COPY)
                nc.vector.tensor_tensor(
                    out=lo, in0=xtb[:], in1=hi, op=mybir.AluOpType.subtract
                )

            # ---- s_col = rowsum(hi+lo) via DR matmuls vs all-ones ----
            ps_s = ps_s_pool.tile([P, T], F32)
            for pi, plane in enumerate((xT_hi, xT_lo)):
                for j in range(NJ):
                    nc.tensor.matmul(
                        ps_s[:], ones2[:], plane[:, 2 * j:2 * j + 2, :],
                        start=(pi == 0 and j == 0),
                        stop=(pi == 1 and j == NJ - 1),
                        perf_mode=DR,
                    )
            nc.vector.tensor_copy(out=s2[0:1, :], in_=ps_s[0:1, :])

            # ---- main o-slab loop ----
            for sl in range(NSLAB):
                if sl > 0 and byte_mode != "dmat":
                    byte_slab(sl)
                wt = wt_pool.tile([P, IT, O_SLAB], U16)
                for kt in range(KT):
                    for b in range(8):
                        it = kt * 8 + b
                        nc.vector.tensor_scalar(
                            out=wt[:, it, :],
                            in0=byteT[:, kt, ts(sl, O_SLAB)],
                            scalar1=SHB - b, scalar2=MASK,
                            op0=mybir.AluOpType.logical_shift_left,
                            op1=mybir.AluOpType.bitwise_and,
                        )
                wt8 = wt[:].bitcast(F8).rearrange(
                    "p it (n two) -> p it n two", two=2
                )
                for tsub in range(TT):
                    ps = ps_mm.tile([P, O_SLAB], F32)
                    nc.tensor.matmul(
                        ps[:], s2[:, ts(tsub, P)], br2[:, ts(sl, O_SLAB)],
                        start=True, stop=False,
                    )
                    for j in range(NJ):
                        for plane in (xT_hi, xT_lo):
                            nc.tensor.matmul(
                                ps[:],
                                plane[:, 2 * j:2 * j + 2, ts(tsub, P)],
                                wt8[:, 2 * j:2 * j + 2, :, 1],
                                start=False,
                                stop=(j == NJ - 1 and plane is xT_lo),
                                perf_mode=DR,
                            )
                    y_sb = y_pool.tile([P, O_SLAB], F32)
                    nc.any.tensor_copy(out=y_sb[:], in_=ps[:])
                    nc.sync.dma_start(
                        y_d[ts(tsub, P), ts(sl, O_SLAB)], y_sb[:]
                    )

    nc.compile()
    return nc


_NC = None


def _get_nc():
    global _NC
    if _NC is None:
        _NC = build()
    return _NC


def run(x, packed_weight, bias, trace=False):
    x = np.ascontiguousarray(np.asarray(x, dtype=np.float32))
    pw = np.ascontiguousarray(np.asarray(packed_weight, dtype=np.uint8))
    bias = np.ascontiguousarray(np.asarray(bias, dtype=np.float32))
    assert x.shape == (B_DIM, S_DIM, I_DIM)
    assert pw.shape == (O_DIM, I_DIM // 8)
    assert bias.shape == (O_DIM,)

    nc = _get_nc()
    xs = x.reshape(T_FULL, I_DIM)
    in_maps = [
        {
            "x": np.ascontiguousarray(xs[c * T_SHARD:(c + 1) * T_SHARD]),
            "pw": pw,
            "bias": bias,
        }
        for c in range(N_CORES)
    ]
    res = run_bass_kernel_spmd(nc, in_maps, list(range(N_CORES)), trace=trace)
    y = np.concatenate(
        [res.results[c][OUT_NAME] for c in range(N_CORES)], axis=0
    )
    return y.reshape(B_DIM, S_DIM, O_DIM), res


def kernel(x, packed_weight, bias):
    y, _ = run(x, packed_weight, bias, trace=False)
    return y


# revision 15
# speedup vs baseline: 1.3504x; 1.0085x over previous
"""BitPackedLinear Trainium2 kernel (8-core SPMD, token-sharded, fp8 DR).

y = x @ W.T + bias, W = unpack_bits(packed_weight) in {-1,+1}, shapes:
  x [2, 2048, 4096] f32, packed_weight [4096, 512] u8, bias [4096] f32.

Sharding: data-parallel over tokens (4096 tokens -> 512/core). Each core
computes y_c = x_c @ W.T + bias for its token shard against the full
weight; the host just concatenates shards.

Device algorithm per core (main GEMM in fp8e4 DoubleRow, 0.5 cyc/row --
2x the bf16 row rate, contracting two 128-i blocks per instruction):
  - x is split exactly as x = hi + lo with hi = fp8(bf16(x)),
    lo = fp8(bf16(x) - hi); W is unpacked to {0, 2.0} and the result
    corrected as y = (hi+lo)@(2B) - rowsum(hi+lo) + bias.  Total split
    error ~1.2e-3 rel (bf16-level).
  - x chunks arrive bf16 (SWDGE cast-DMA; kt=0 via HWDGE f32 + DVE cast
    to dodge the slow Q7 descriptor-gen start), bit-sliced layout.
  - PE transposes x per [128,128] bf16 tile into per-kt groups, then one
    ACT copy casts the group to the fp8 hi plane and one DVE subtract
    produces the fp8 lo plane: xT_hi/xT_lo [128 i-part, 32 it, 512 tok],
    tokens contiguous (the dual-fp8 Ldweights ISA check requires a
    unit-stride stationary m dim; the moving ifmap tolerates stride 2).
  - byteT[k',kt,o] = pw[o,128kt+k'] via u8->u16 cast-DMA bounce through
    DRAM + one XBAR transpose-DMA per kt (no PE/DVE involvement).
  - W unpack stays 1 op/tile: wt16 = (u16(byte) << (14-b)) & 0x4000
    puts fp8 {0, 2.0} in the HIGH byte; the DR matmul reads odd bytes
    ([p, 2, 512] pair-stride 1024, n-stride 2 -- validated on hw).
  - Main DR matmuls per (o-slab, token-tile), for j in 0..16, term in
    (hi, lo): psum += plane[:, 2j:2j+2, t].T (x) wt8[:, 2j:2j+2, :].
  - s_col = rowsum(hi+lo) computed EXACTLY by 32 DR matmuls against an
    all-ones stationary -> psum row -> SBUF f32r row.
  - bias + (-s_col) enter each psum via ONE rank-2 f32r matmul
    (k=0: s_col x -1, k=1: ones x bias) with start=True; epilogue is a
    plain psum->SBUF copy + DMA.
"""
import sys

sys.path.insert(0, "/opt/trn_rl_repo")
from contextlib import ExitStack

import numpy as np

import concourse.tile as tile
from concourse import bacc, mybir
from concourse.bass import ts
from concourse.bass_utils import run_bass_kernel_spmd
from concourse.masks import make_identity

F32 = mybir.dt.float32
F32R = mybir.dt.float32r
BF16 = mybir.dt.bfloat16
F8 = mybir.dt.float8e4
U8 = mybir.dt.uint8
U16 = mybir.dt.uint16
P = 128
DR = mybir.MatmulPerfMode.DoubleRow
ACT_COPY = mybir.ActivationFunctionType.Copy

N_CORES = 8
B_DIM, S_DIM, I_DIM, O_DIM = 2, 2048, 4096, 4096
T_FULL = B_DIM * S_DIM          # 4096 tokens
T_SHARD = T_FULL // N_CORES     # 512 tokens per core
OUT_NAME = "y"
OUT_SHAPE = (T_SHARD, O_DIM)


def build(T=T_SHARD, I=I_DIM, O=O_DIM, O_SLAB=512, n_cores=N_CORES,
          byte_mode="dmat"):
    assert I % 1024 == 0 and T % P == 0 and O % P == 0 and O % O_SLAB == 0
    KT = I // 1024          # 128-byte groups along i (4)
    IT = KT * 8             # bit-sliced i-tiles (32)
    NJ = IT // 2            # DR i-tile pairs (16)
    TT = T // P             # token tiles (4)
    K = I // 8              # packed bytes per weight row (512)
    NSLAB = O // O_SLAB
    OSL_T = O_SLAB // P
    SHB, MASK = 14, 0x4000  # unpack: fp8 {0,2.0} pattern in the HIGH byte

    nc = bacc.Bacc("TRN2", target_bir_lowering=False, debug=False,
                   num_devices=n_cores)
    x_d = nc.dram_tensor("x", [T, I], F32, kind="ExternalInput").ap()
    pw_d = nc.dram_tensor("pw", [O, K], U8, kind="ExternalInput").ap()
    bias_d = nc.dram_tensor("bias", [O], F32, kind="ExternalInput").ap()
    y_d = nc.dram_tensor(OUT_NAME, [T, O], F32, kind="ExternalOutput").ap()
    pw16_d = nc.dram_tensor("pw16", [O, K], U16).ap()

    with tile.TileContext(nc) as tc:
        with ExitStack() as ctx:
            const = ctx.enter_context(tc.tile_pool(name="const", bufs=1))
            persist = ctx.enter_context(tc.tile_pool(name="persist", bufs=1))
            stage = ctx.enter_context(tc.tile_pool(name="stage", bufs=1))

            ident_bf = const.tile([P, P], BF16)
            make_identity(nc, ident_bf[:])
            ones2 = const.tile([P, 2, P], F8)
            nc.vector.memset(ones2[:], 1.0)

            # rank-2 psum-init operands; engine-written rows sit at
            # partition 0 (engine APs must start on a quarter boundary),
            # partition-1 rows are DMA-filled.
            # ps = s2[0,t]*br2[0,o] + s2[1,t]*br2[1,o]
            #    = s_col[t]*(-1)    + 1*bias[o]
            s2 = const.tile([2, T], F32R)    # row0 = s_col, row1 = ones
            br2 = const.tile([2, O], F32R)   # row0 = -ones, row1 = bias

            byteT = persist.tile([P, KT, O], U16)
            xT_hi = persist.tile([P, IT, T], F8)
            xT_lo = persist.tile([P, IT, T], F8)

            x32_pool = ctx.enter_context(tc.tile_pool(name="x32", bufs=2))
            xn_pool = ctx.enter_context(tc.tile_pool(name="xnat", bufs=10))
            xtb_pool = ctx.enter_context(tc.tile_pool(name="xtb", bufs=2))
            pk_pool = ctx.enter_context(tc.tile_pool(name="pk", bufs=2))
            pkbf_pool = ctx.enter_context(tc.tile_pool(name="pkbf", bufs=2))
            wt_pool = ctx.enter_context(tc.tile_pool(name="wt", bufs=2))
            y_pool = ctx.enter_context(tc.tile_pool(name="ysb", bufs=3))
            ps_tr = ctx.enter_context(
                tc.tile_pool(name="ps_tr", bufs=3, space="PSUM")
            )
            ps_mm = ctx.enter_context(
                tc.tile_pool(name="ps_mm", bufs=4, space="PSUM")
            )
            ps_s_pool = ctx.enter_context(
                tc.tile_pool(name="ps_s", bufs=1, space="PSUM")
            )


            def emit_bias_stage():
                nc.sync.dma_start(
                    br2[1:2, :],
                    bias_d.bitcast(F32R).rearrange("(b o) -> b o", b=1),
                )
                cst = stage.tile([1, T], F32)
                nc.vector.memset(cst[:], -1.0)
                nc.vector.tensor_copy(
                    out=br2[0:1, :],
                    in_=cst[:, :1].broadcast_to([1, O]),
                )
                ones_row = stage.tile([1, T], F32R)
                nc.vector.memset(cst[:], 1.0)
                nc.vector.tensor_copy(out=ones_row[:], in_=cst[:])
                nc.sync.dma_start(s2[1:2, :], ones_row[:])

            def byte_slab(sl):
                """Fill byteT[:, :, sl*O_SLAB:(sl+1)*O_SLAB] via PE."""
                pw_ap = pw_d.rearrange("(ot p) k -> p ot k", p=P)
                pk = pk_pool.tile([P, OSL_T, K], U8)
                nc.sync.dma_start(pk[:], pw_ap[:, ts(sl, OSL_T), :])
                for otl in range(OSL_T):
                    ot = sl * OSL_T + otl
                    pkbf = pkbf_pool.tile([P, K], BF16)
                    nc.any.tensor_copy(out=pkbf[:], in_=pk[:, otl, :])
                    for kt in range(KT):
                        ps = ps_tr.tile([P, P], BF16, tag="tr_ps")
                        nc.tensor.transpose(ps[:], pkbf[:, ts(kt, P)], ident_bf[:])
                        nc.any.tensor_copy(out=byteT[:, kt, ts(ot, P)], in_=ps[:])

            # ---- byte path (per-kt cast then XBAR, so the first
            # unpackable column arrives early) ----
            if byte_mode == "dmat":
                nc.gpsimd.dma_start(out=pw16_d[:], in_=pw_d[:])
                for kt in range(KT):
                    nc.sync.dma_start_transpose(
                        byteT[:, kt, :], pw16_d[:, ts(kt, P)]
                    )
            else:
                byte_slab(0)

            emit_bias_stage()

            # PE warm-up: ~24 back-to-back identity transposes ramp the
            # p-state to full clock while the first DMAs land.
            for _ in range(24):
                ps_warm = ps_tr.tile([P, P], BF16, tag="tr_ps")
                nc.tensor.transpose(ps_warm[:], ident_bf[:], ident_bf[:])

            # ---- x chunks, kt-major; kt=0 via HWDGE f32 + DVE cast ----
            xns = {}
            for kt in range(KT):
                for tt in range(TT):
                    src_ap = x_d[ts(tt, P), ts(kt, 1024)].rearrange(
                        "p (k b) -> p k b", b=8
                    )
                    xn = xn_pool.tile([P, P, 8], BF16, tag="xn16")
                    if kt == 0:
                        x32 = x32_pool.tile([P, P, 8], F32)
                        nc.sync.dma_start(x32[:], src_ap)
                        nc.vector.tensor_copy(out=xn[:], in_=x32[:])
                    else:
                        nc.gpsimd.dma_start(xn[:], src_ap)
                    xns[kt, tt] = xn

            # ---- transpose (bf16) per kt group, then split hi/lo fp8 ----
            for kt in range(KT):
                xtb = xtb_pool.tile([P, 8, T], BF16)
                for b in range(8):
                    for tt in range(TT):
                        ps = ps_tr.tile([P, P], BF16, tag="tr_ps")
                        nc.tensor.transpose(
                            ps[:], xns[kt, tt][:, :, b], ident_bf[:]
                        )
                        dst = xtb[:, b, ts(tt, P)]
                        if (b * TT + tt) % 2 == 0:
                            nc.vector.tensor_copy(out=dst, in_=ps[:])
                        else:
                            nc.scalar.activation(out=dst, in_=ps[:], func=ACT_COPY)
                hi = xT_hi[:, ts(kt, 8), :]
                lo = xT_lo[:, ts(kt, 8), :]
                nc.scalar.activation(out=hi, in_=xtb[:], func=ACT_COPY)
                nc.vector.tensor_tensor(
                    out=lo, in0=xtb[:], in1=hi, op=mybir.AluOpType.subtract
                )

            # ---- s_col = rowsum(hi+lo) via DR matmuls vs all-ones ----
            ps_s = ps_s_pool.tile([P, T], F32)
            for pi, plane in enumerate((xT_hi, xT_lo)):
                for j in range(NJ):
                    nc.tensor.matmul(
                        ps_s[:], ones2[:], plane[:, 2 * j:2 * j + 2, :],
                        start=(pi == 0 and j == 0),
                        stop=(pi == 1 and j == NJ - 1),
                        perf_mode=DR,
                    )
            nc.vector.tensor_copy(out=s2[0:1, :], in_=ps_s[0:1, :])

            # ---- main o-slab loop ----
            for sl in range(NSLAB):
                if sl > 0 and byte_mode != "dmat":
                    byte_slab(sl)
                wt = wt_pool.tile([P, IT, O_SLAB], U16)
                for kt in range(KT):
                    for b in range(8):
                        it = kt * 8 + b
                        nc.vector.tensor_scalar(
                            out=wt[:, it, :],
                            in0=byteT[:, kt, ts(sl, O_SLAB)],
                            scalar1=SHB - b, scalar2=MASK,
                            op0=mybir.AluOpType.logical_shift_left,
                            op1=mybir.AluOpType.bitwise_and,
                        )
                wt8 = wt[:].bitcast(F8).rearrange(
                    "p it (n two) -> p it n two", two=2
                )
                for tsub in range(TT):
                    ps = ps_mm.tile([P, O_SLAB], F32)
                    for j in range(NJ):
                        for plane in (xT_hi, xT_lo):
                            nc.tensor.matmul(
                                ps[:],
                                plane[:, 2 * j:2 * j + 2, ts(tsub, P)],
                                wt8[:, 2 * j:2 * j + 2, :, 1],
                                start=(j == 0 and plane is xT_hi),
                                stop=False,
                                perf_mode=DR,
                            )
                    # bias - s_col enters last so the DR stream above can
                    # start before s_col is known
                    nc.tensor.matmul(
                        ps[:], s2[:, ts(tsub, P)], br2[:, ts(sl, O_SLAB)],
                        start=False, stop=True,
                    )
                    y_sb = y_pool.tile([P, O_SLAB], F32)
                    nc.scalar.activation(out=y_sb[:], in_=ps[:], func=ACT_COPY)
                    nc.sync.dma_start(
                        y_d[ts(tsub, P), ts(sl, O_SLAB)], y_sb[:]
                    )

    nc.compile()
    return nc


_NC = None


def _get_nc():
    global _NC
    if _NC is None:
        _NC = build()
    return _NC


def run(x, packed_weight, bias, trace=False):
    x = np.ascontiguousarray(np.asarray(x, dtype=np.float32))
    pw = np.ascontiguousarray(np.asarray(packed_weight, dtype=np.uint8))
    bias = np.ascontiguousarray(np.asarray(bias, dtype=np.float32))
    assert x.shape == (B_DIM, S_DIM, I_DIM)
    assert pw.shape == (O_DIM, I_DIM // 8)
    assert bias.shape == (O_DIM,)

    nc = _get_nc()
    xs = x.reshape(T_FULL, I_DIM)
    in_maps = [
        {
            "x": np.ascontiguousarray(xs[c * T_SHARD:(c + 1) * T_SHARD]),
            "pw": pw,
            "bias": bias,
        }
        for c in range(N_CORES)
    ]
    res = run_bass_kernel_spmd(nc, in_maps, list(range(N_CORES)), trace=trace)
    y = np.concatenate(
        [res.results[c][OUT_NAME] for c in range(N_CORES)], axis=0
    )
    return y.reshape(B_DIM, S_DIM, O_DIM), res


def kernel(x, packed_weight, bias):
    y, _ = run(x, packed_weight, bias, trace=False)
    return y


# revision 21
# speedup vs baseline: 1.3722x; 1.0162x over previous
"""BitPackedLinear Trainium2 kernel (8-core SPMD, token-sharded, fp8 DR).

y = x @ W.T + bias, W = unpack_bits(packed_weight) in {-1,+1}, shapes:
  x [2, 2048, 4096] f32, packed_weight [4096, 512] u8, bias [4096] f32.

Sharding: data-parallel over tokens (4096 tokens -> 512/core). Each core
computes y_c = x_c @ W.T + bias for its token shard against the full
weight; the host just concatenates shards.

Device algorithm per core (main GEMM in fp8e4 DoubleRow, 0.5 cyc/row --
2x the bf16 row rate, contracting two 128-i blocks per instruction):
  - x is split exactly as x = hi + lo with hi = fp8(bf16(x)),
    lo = fp8(bf16(x) - hi); W is unpacked to {0, 2.0} and the result
    corrected as y = (hi+lo)@(2B) - rowsum(hi+lo) + bias.  Total split
    error ~1.2e-3 rel (bf16-level).
  - x chunks arrive bf16 (SWDGE cast-DMA; kt=0 via HWDGE f32 + DVE cast
    to dodge the slow Q7 descriptor-gen start), bit-sliced layout.
  - PE transposes x per [128,128] bf16 tile into per-kt groups, then one
    ACT copy casts the group to the fp8 hi plane and one DVE subtract
    produces the fp8 lo plane: xT_hi/xT_lo [128 i-part, 32 it, 512 tok],
    tokens contiguous (the dual-fp8 Ldweights ISA check requires a
    unit-stride stationary m dim; the moving ifmap tolerates stride 2).
  - byteT[k',kt,o] = pw[o,128kt+k'] via u8->u16 cast-DMA bounce through
    DRAM + one XBAR transpose-DMA per kt (no PE/DVE involvement).
  - W unpack stays 1 op/tile: wt16 = (u16(byte) << (14-b)) & 0x4000
    puts fp8 {0, 2.0} in the HIGH byte; the DR matmul reads odd bytes
    ([p, 2, 512] pair-stride 1024, n-stride 2 -- validated on hw).
  - Main DR matmuls per (o-slab, token-tile), for j in 0..16, term in
    (hi, lo): psum += plane[:, 2j:2j+2, t].T (x) wt8[:, 2j:2j+2, :].
  - s_col = rowsum(hi+lo) computed EXACTLY by 32 DR matmuls against an
    all-ones stationary -> psum row -> SBUF f32r row.
  - bias + (-s_col) enter each psum via ONE rank-2 f32r matmul
    (k=0: s_col x -1, k=1: ones x bias) with start=True; epilogue is a
    plain psum->SBUF copy + DMA.
"""
import sys

sys.path.insert(0, "/opt/trn_rl_repo")
from contextlib import ExitStack

import numpy as np

import concourse.tile as tile
from concourse import bacc, mybir
from concourse.bass import ts
from concourse.bass_utils import run_bass_kernel_spmd
from concourse.masks import make_identity

F32 = mybir.dt.float32
F32R = mybir.dt.float32r
BF16 = mybir.dt.bfloat16
F8 = mybir.dt.float8e4
U8 = mybir.dt.uint8
U16 = mybir.dt.uint16
P = 128
DR = mybir.MatmulPerfMode.DoubleRow
ACT_COPY = mybir.ActivationFunctionType.Copy

N_CORES = 8
B_DIM, S_DIM, I_DIM, O_DIM = 2, 2048, 4096, 4096
T_FULL = B_DIM * S_DIM          # 4096 tokens
T_SHARD = T_FULL // N_CORES     # 512 tokens per core
OUT_NAME = "y"
OUT_SHAPE = (T_SHARD, O_DIM)


def build(T=T_SHARD, I=I_DIM, O=O_DIM, O_SLAB=512, n_cores=N_CORES,
          byte_mode="dmat"):
    assert I % 1024 == 0 and T % P == 0 and O % P == 0 and O % O_SLAB == 0
    KT = I // 1024          # 128-byte groups along i (4)
    IT = KT * 8             # bit-sliced i-tiles (32)
    NJ = IT // 2            # DR i-tile pairs (16)
    TT = T // P             # token tiles (4)
    K = I // 8              # packed bytes per weight row (512)
    NSLAB = O // O_SLAB
    OSL_T = O_SLAB // P
    SHB, MASK = 14, 0x4000  # unpack: fp8 {0,2.0} pattern in the HIGH byte

    nc = bacc.Bacc("TRN2", target_bir_lowering=False, debug=False,
                   num_devices=n_cores)
    x_d = nc.dram_tensor("x", [T, I], F32, kind="ExternalInput").ap()
    pw_d = nc.dram_tensor("pw", [O, K], U8, kind="ExternalInput").ap()
    bias_d = nc.dram_tensor("bias", [O], F32, kind="ExternalInput").ap()
    y_d = nc.dram_tensor(OUT_NAME, [T, O], F32, kind="ExternalOutput").ap()
    pw16_d = nc.dram_tensor("pw16", [O, K], U16).ap()

    with tile.TileContext(nc) as tc:
        with ExitStack() as ctx:
            const = ctx.enter_context(tc.tile_pool(name="const", bufs=1))
            persist = ctx.enter_context(tc.tile_pool(name="persist", bufs=1))
            stage = ctx.enter_context(tc.tile_pool(name="stage", bufs=1))

            ident_bf = const.tile([P, P], BF16)
            make_identity(nc, ident_bf[:])
            ones2 = const.tile([P, 2, P], F8)
            nc.vector.memset(ones2[:], 1.0)

            # rank-2 psum-init operands; engine-written rows sit at
            # partition 0 (engine APs must start on a quarter boundary),
            # partition-1 rows are DMA-filled.
            # ps = s2[0,t]*br2[0,o] + s2[1,t]*br2[1,o]
            #    = s_col[t]*(-1)    + 1*bias[o]
            s2 = const.tile([2, T], F32R)    # row0 = s_col, row1 = ones
            br2 = const.tile([2, O], F32R)   # row0 = -ones, row1 = bias

            byteT = persist.tile([P, KT, O], U16)
            xT_hi = persist.tile([P, IT, T], F8)
            xT_lo = persist.tile([P, IT, T], F8)

            x32_pool = ctx.enter_context(tc.tile_pool(name="x32", bufs=2))
            xn_pool = ctx.enter_context(tc.tile_pool(name="xnat", bufs=10))
            xtb_pool = ctx.enter_context(tc.tile_pool(name="xtb", bufs=2))
            pk_pool = ctx.enter_context(tc.tile_pool(name="pk", bufs=2))
            pkbf_pool = ctx.enter_context(tc.tile_pool(name="pkbf", bufs=2))
            wt_pool = ctx.enter_context(tc.tile_pool(name="wt", bufs=2))
            y_pool = ctx.enter_context(tc.tile_pool(name="ysb", bufs=3))
            ps_tr = ctx.enter_context(
                tc.tile_pool(name="ps_tr", bufs=3, space="PSUM")
            )
            ps_mm = ctx.enter_context(
                tc.tile_pool(name="ps_mm", bufs=4, space="PSUM")
            )
            ps_s_pool = ctx.enter_context(
                tc.tile_pool(name="ps_s", bufs=1, space="PSUM")
            )


            def emit_bias_stage():
                nc.sync.dma_start(
                    br2[1:2, :],
                    bias_d.bitcast(F32R).rearrange("(b o) -> b o", b=1),
                )
                cst = stage.tile([1, T], F32)
                nc.vector.memset(cst[:], -1.0)
                nc.vector.tensor_copy(
                    out=br2[0:1, :],
                    in_=cst[:, :1].broadcast_to([1, O]),
                )
                ones_row = stage.tile([1, T], F32R)
                nc.vector.memset(cst[:], 1.0)
                nc.vector.tensor_copy(out=ones_row[:], in_=cst[:])
                nc.sync.dma_start(s2[1:2, :], ones_row[:])

            def byte_slab(sl):
                """Fill byteT[:, :, sl*O_SLAB:(sl+1)*O_SLAB] via PE."""
                pw_ap = pw_d.rearrange("(ot p) k -> p ot k", p=P)
                pk = pk_pool.tile([P, OSL_T, K], U8)
                nc.sync.dma_start(pk[:], pw_ap[:, ts(sl, OSL_T), :])
                for otl in range(OSL_T):
                    ot = sl * OSL_T + otl
                    pkbf = pkbf_pool.tile([P, K], BF16)
                    nc.any.tensor_copy(out=pkbf[:], in_=pk[:, otl, :])
                    for kt in range(KT):
                        ps = ps_tr.tile([P, P], BF16, tag="tr_ps")
                        nc.tensor.transpose(ps[:], pkbf[:, ts(kt, P)], ident_bf[:])
                        nc.any.tensor_copy(out=byteT[:, kt, ts(ot, P)], in_=ps[:])

            # ---- byte path: slab-0 bytes first (tiny), so the first
            # weight tile is unpackable within a few us ----
            if byte_mode == "dmat":
                nc.gpsimd.dma_start(out=pw16_d[:], in_=pw_d[:])
                for kt in range(KT):
                    nc.sync.dma_start_transpose(
                        byteT[:, kt, :], pw16_d[:, ts(kt, P)]
                    )
            else:
                byte_slab(0)

            emit_bias_stage()

            # PE warm-up: ~24 back-to-back identity transposes ramp the
            # p-state to full clock while the first DMAs land.
            for _ in range(24):
                ps_warm = ps_tr.tile([P, P], BF16, tag="tr_ps")
                nc.tensor.transpose(ps_warm[:], ident_bf[:], ident_bf[:])

            # slab-0 unpack leads the DVE queue (only needs slab-0 bytes)
            def unpack_slab(sl):
                wt = wt_pool.tile([P, IT, O_SLAB], U16)
                for kt in range(KT):
                    for b in range(8):
                        it = kt * 8 + b
                        nc.vector.tensor_scalar(
                            out=wt[:, it, :],
                            in0=byteT[:, kt, ts(sl, O_SLAB)],
                            scalar1=SHB - b, scalar2=MASK,
                            op0=mybir.AluOpType.logical_shift_left,
                            op1=mybir.AluOpType.bitwise_and,
                        )
                return wt

            wt0 = unpack_slab(0)

            # ---- x chunks, all via SWDGE cast-DMA (Pool queue leads
            # with the small slab-0 pw16 cast only) ----
            xns = {}
            for kt in range(KT):
                for tt in range(TT):
                    src_ap = x_d[ts(tt, P), ts(kt, 1024)].rearrange(
                        "p (k b) -> p k b", b=8
                    )
                    xn = xn_pool.tile([P, P, 8], BF16, tag="xn16")
                    if kt == 0:
                        x32 = x32_pool.tile([P, P, 8], F32)
                        nc.sync.dma_start(x32[:], src_ap)
                        nc.vector.tensor_copy(out=xn[:], in_=x32[:])
                    else:
                        nc.gpsimd.dma_start(xn[:], src_ap)
                    xns[kt, tt] = xn

            # ---- transpose (bf16) per kt group, then split hi/lo fp8
            # per (kt, tt) so the tail of the cast chain is short ----
            for kt in range(KT):
                xtb = xtb_pool.tile([P, 8, T], BF16)
                for tt in range(TT):
                    for b in range(8):
                        ps = ps_tr.tile([P, P], BF16, tag="tr_ps")
                        nc.tensor.transpose(
                            ps[:], xns[kt, tt][:, :, b], ident_bf[:]
                        )
                        dst = xtb[:, b, ts(tt, P)]
                        if (b * TT + tt) % 2 == 0:
                            nc.vector.tensor_copy(out=dst, in_=ps[:])
                        else:
                            nc.scalar.activation(out=dst, in_=ps[:], func=ACT_COPY)
                    hi = xT_hi[:, ts(kt, 8), ts(tt, P)]
                    lo = xT_lo[:, ts(kt, 8), ts(tt, P)]
                    src = xtb[:, :, ts(tt, P)]
                    nc.scalar.activation(out=hi, in_=src, func=ACT_COPY)
                    nc.vector.tensor_tensor(
                        out=lo, in0=src, in1=hi, op=mybir.AluOpType.subtract
                    )

            # ---- main o-slab loop (s_col DRs slot in after the first
            # group's DR stream; each group's rank-2 finisher needs s2) ----
            def emit_s_col():
                ps_s = ps_s_pool.tile([P, T], F32)
                for pi, plane in enumerate((xT_hi, xT_lo)):
                    for j in range(NJ):
                        nc.tensor.matmul(
                            ps_s[:], ones2[:], plane[:, 2 * j:2 * j + 2, :],
                            start=(pi == 0 and j == 0),
                            stop=(pi == 1 and j == NJ - 1),
                            perf_mode=DR,
                        )
                nc.vector.tensor_copy(out=s2[0:1, :], in_=ps_s[0:1, :])

            emit_s_col()

            for sl in range(NSLAB):
                if sl > 0 and byte_mode != "dmat":
                    byte_slab(sl)
                wt = wt0 if sl == 0 else unpack_slab(sl)
                wt8 = wt[:].bitcast(F8).rearrange(
                    "p it (n two) -> p it n two", two=2
                )
                for tsub in range(TT):
                    ps = ps_mm.tile([P, O_SLAB], F32)
                    for j in range(NJ):
                        for plane in (xT_hi, xT_lo):
                            nc.tensor.matmul(
                                ps[:],
                                plane[:, 2 * j:2 * j + 2, ts(tsub, P)],
                                wt8[:, 2 * j:2 * j + 2, :, 1],
                                start=(j == 0 and plane is xT_hi),
                                stop=False,
                                perf_mode=DR,
                            )
                    # bias - s_col enters last so the DR stream above can
                    # start before s_col is known
                    nc.tensor.matmul(
                        ps[:], s2[:, ts(tsub, P)], br2[:, ts(sl, O_SLAB)],
                        start=False, stop=True,
                    )
                    y_sb = y_pool.tile([P, O_SLAB], F32)
                    nc.scalar.activation(out=y_sb[:], in_=ps[:], func=ACT_COPY)
                    nc.sync.dma_start(
                        y_d[ts(tsub, P), ts(sl, O_SLAB)], y_sb[:]
                    )

    nc.compile()
    return nc


_NC = None


def _get_nc():
    global _NC
    if _NC is None:
        _NC = build()
    return _NC


def run(x, packed_weight, bias, trace=False):
    x = np.ascontiguousarray(np.asarray(x, dtype=np.float32))
    pw = np.ascontiguousarray(np.asarray(packed_weight, dtype=np.uint8))
    bias = np.ascontiguousarray(np.asarray(bias, dtype=np.float32))
    assert x.shape == (B_DIM, S_DIM, I_DIM)
    assert pw.shape == (O_DIM, I_DIM // 8)
    assert bias.shape == (O_DIM,)

    nc = _get_nc()
    xs = x.reshape(T_FULL, I_DIM)
    in_maps = [
        {
            "x": np.ascontiguousarray(xs[c * T_SHARD:(c + 1) * T_SHARD]),
            "pw": pw,
            "bias": bias,
        }
        for c in range(N_CORES)
    ]
    res = run_bass_kernel_spmd(nc, in_maps, list(range(N_CORES)), trace=trace)
    y = np.concatenate(
        [res.results[c][OUT_NAME] for c in range(N_CORES)], axis=0
    )
    return y.reshape(B_DIM, S_DIM, O_DIM), res


def kernel(x, packed_weight, bias):
    y, _ = run(x, packed_weight, bias, trace=False)
    return y


# revision 24
# speedup vs baseline: 1.4050x; 1.0239x over previous
"""BitPackedLinear Trainium2 kernel (8-core SPMD, token-sharded, fp8 DR).

y = x @ W.T + bias, W = unpack_bits(packed_weight) in {-1,+1}, shapes:
  x [2, 2048, 4096] f32, packed_weight [4096, 512] u8, bias [4096] f32.

Sharding: data-parallel over tokens (4096 tokens -> 512/core). Each core
computes y_c = x_c @ W.T + bias for its token shard against the full
weight; the host just concatenates shards.

Device algorithm per core (main GEMM in fp8e4 DoubleRow, 0.5 cyc/row --
2x the bf16 row rate, contracting two 128-i blocks per instruction):
  - x is split exactly as x = hi + lo with hi = fp8(bf16(x)),
    lo = fp8(bf16(x) - hi); W is unpacked to {0, 2.0} and the result
    corrected as y = (hi+lo)@(2B) - rowsum(hi+lo) + bias.  Total split
    error ~1.2e-3 rel (bf16-level).
  - x chunks arrive bf16 (SWDGE cast-DMA; kt=0 via HWDGE f32 + DVE cast
    to dodge the slow Q7 descriptor-gen start), bit-sliced layout.
  - PE transposes x per [128,128] bf16 tile into per-kt groups, then one
    ACT copy casts the group to the fp8 hi plane and one DVE subtract
    produces the fp8 lo plane: xT_hi/xT_lo [128 i-part, 32 it, 512 tok],
    tokens contiguous (the dual-fp8 Ldweights ISA check requires a
    unit-stride stationary m dim; the moving ifmap tolerates stride 2).
  - byteT[k',kt,o] = pw[o,128kt+k'] via u8->u16 cast-DMA bounce through
    DRAM + one XBAR transpose-DMA per kt (no PE/DVE involvement).
  - W unpack stays 1 op/tile: wt16 = (u16(byte) << (14-b)) & 0x4000
    puts fp8 {0, 2.0} in the HIGH byte; the DR matmul reads odd bytes
    ([p, 2, 512] pair-stride 1024, n-stride 2 -- validated on hw).
  - Main DR matmuls per (o-slab, token-tile), for j in 0..16, term in
    (hi, lo): psum += plane[:, 2j:2j+2, t].T (x) wt8[:, 2j:2j+2, :].
  - s_col = rowsum(hi+lo) computed EXACTLY by 32 DR matmuls against an
    all-ones stationary -> psum row -> SBUF f32r row.
  - bias + (-s_col) enter each psum via ONE rank-2 f32r matmul
    (k=0: s_col x -1, k=1: ones x bias) with start=True; epilogue is a
    plain psum->SBUF copy + DMA.
"""
import sys

sys.path.insert(0, "/opt/trn_rl_repo")
from contextlib import ExitStack

import numpy as np

import concourse.tile as tile
from concourse import bacc, mybir
from concourse.bass import ts
from concourse.bass_utils import run_bass_kernel_spmd
from concourse.masks import make_identity

F32 = mybir.dt.float32
F32R = mybir.dt.float32r
BF16 = mybir.dt.bfloat16
F8 = mybir.dt.float8e4
U8 = mybir.dt.uint8
U16 = mybir.dt.uint16
P = 128
DR = mybir.MatmulPerfMode.DoubleRow
ACT_COPY = mybir.ActivationFunctionType.Copy

N_CORES = 8
B_DIM, S_DIM, I_DIM, O_DIM = 2, 2048, 4096, 4096
T_FULL = B_DIM * S_DIM          # 4096 tokens
T_SHARD = T_FULL // N_CORES     # 512 tokens per core
OUT_NAME = "y"
OUT_SHAPE = (T_SHARD, O_DIM)


def build(T=T_SHARD, I=I_DIM, O=O_DIM, O_SLAB=512, n_cores=N_CORES,
          byte_mode="dmat"):
    assert I % 1024 == 0 and T % P == 0 and O % P == 0 and O % O_SLAB == 0
    KT = I // 1024          # 128-byte groups along i (4)
    IT = KT * 8             # bit-sliced i-tiles (32)
    NJ = IT // 2            # DR i-tile pairs (16)
    TT = T // P             # token tiles (4)
    K = I // 8              # packed bytes per weight row (512)
    NSLAB = O // O_SLAB
    OSL_T = O_SLAB // P
    SHB, MASK = 14, 0x4000  # unpack: fp8 {0,2.0} pattern in the HIGH byte

    nc = bacc.Bacc("TRN2", target_bir_lowering=False, debug=False,
                   num_devices=n_cores)
    x_d = nc.dram_tensor("x", [T, I], F32, kind="ExternalInput").ap()
    pw_d = nc.dram_tensor("pw", [O, K], U8, kind="ExternalInput").ap()
    bias_d = nc.dram_tensor("bias", [O], F32, kind="ExternalInput").ap()
    y_d = nc.dram_tensor(OUT_NAME, [T, O], F32, kind="ExternalOutput").ap()
    pw16_d = nc.dram_tensor("pw16", [O, K], U16).ap()

    with tile.TileContext(nc) as tc:
        with ExitStack() as ctx:
            const = ctx.enter_context(tc.tile_pool(name="const", bufs=1))
            persist = ctx.enter_context(tc.tile_pool(name="persist", bufs=1))
            stage = ctx.enter_context(tc.tile_pool(name="stage", bufs=1))

            ident_bf = const.tile([P, P], BF16)
            make_identity(nc, ident_bf[:])
            ones2 = const.tile([P, 2, P], F8)
            nc.vector.memset(ones2[:], 1.0)

            # rank-2 psum-init operands; engine-written rows sit at
            # partition 0 (engine APs must start on a quarter boundary),
            # partition-1 rows are DMA-filled.
            # ps = s2[0,t]*br2[0,o] + s2[1,t]*br2[1,o]
            #    = s_col[t]*(-1)    + 1*bias[o]
            s2 = const.tile([2, T], F32R)    # row0 = s_col, row1 = ones
            br2 = const.tile([2, O], F32R)   # row0 = -ones, row1 = bias

            byteT = persist.tile([P, KT, O], U16)
            xT_hi = persist.tile([P, IT, T], F8)
            xT_lo = persist.tile([P, IT, T], F8)

            x32_pool = ctx.enter_context(tc.tile_pool(name="x32", bufs=2))
            xn_pool = ctx.enter_context(tc.tile_pool(name="xnat", bufs=10))
            xtb_pool = ctx.enter_context(tc.tile_pool(name="xtb", bufs=2))
            pk_pool = ctx.enter_context(tc.tile_pool(name="pk", bufs=2))
            pkbf_pool = ctx.enter_context(tc.tile_pool(name="pkbf", bufs=2))
            wt_pool = ctx.enter_context(tc.tile_pool(name="wt", bufs=2))
            y_pool = ctx.enter_context(tc.tile_pool(name="ysb", bufs=3))
            ps_tr = ctx.enter_context(
                tc.tile_pool(name="ps_tr", bufs=3, space="PSUM")
            )
            ps_mm = ctx.enter_context(
                tc.tile_pool(name="ps_mm", bufs=4, space="PSUM")
            )
            ps_s_pool = ctx.enter_context(
                tc.tile_pool(name="ps_s", bufs=1, space="PSUM")
            )


            def emit_bias_stage():
                nc.sync.dma_start(
                    br2[1:2, :],
                    bias_d.bitcast(F32R).rearrange("(b o) -> b o", b=1),
                )
                cst = stage.tile([1, T], F32)
                nc.vector.memset(cst[:], -1.0)
                nc.vector.tensor_copy(
                    out=br2[0:1, :],
                    in_=cst[:, :1].broadcast_to([1, O]),
                )
                ones_row = stage.tile([1, T], F32R)
                nc.vector.memset(cst[:], 1.0)
                nc.vector.tensor_copy(out=ones_row[:], in_=cst[:])
                nc.sync.dma_start(s2[1:2, :], ones_row[:])

            def byte_slab(sl):
                """Fill byteT[:, :, sl*O_SLAB:(sl+1)*O_SLAB] via PE."""
                pw_ap = pw_d.rearrange("(ot p) k -> p ot k", p=P)
                pk = pk_pool.tile([P, OSL_T, K], U8)
                nc.sync.dma_start(pk[:], pw_ap[:, ts(sl, OSL_T), :])
                for otl in range(OSL_T):
                    ot = sl * OSL_T + otl
                    pkbf = pkbf_pool.tile([P, K], BF16)
                    nc.any.tensor_copy(out=pkbf[:], in_=pk[:, otl, :])
                    for kt in range(KT):
                        ps = ps_tr.tile([P, P], BF16, tag="tr_ps")
                        nc.tensor.transpose(ps[:], pkbf[:, ts(kt, P)], ident_bf[:])
                        nc.any.tensor_copy(out=byteT[:, kt, ts(ot, P)], in_=ps[:])

            # ---- byte path: slab 0 via the PE path (ready in a few us,
            # no DMA-engine contention); slabs 1-7 via pw16 bounce + XBAR
            # transpose-DMAs emitted after the x chunks ----
            byte_slab(0)

            emit_bias_stage()

            # PE warm-up: ~24 back-to-back identity transposes ramp the
            # p-state to full clock while the first DMAs land.
            for _ in range(24):
                ps_warm = ps_tr.tile([P, P], BF16, tag="tr_ps")
                nc.tensor.transpose(ps_warm[:], ident_bf[:], ident_bf[:])

            # slab-0/1 unpacks lead the DVE queue
            def unpack_slab(sl):
                wt = wt_pool.tile([P, IT, O_SLAB], U16)
                for kt in range(KT):
                    for b in range(8):
                        it = kt * 8 + b
                        nc.vector.tensor_scalar(
                            out=wt[:, it, :],
                            in0=byteT[:, kt, ts(sl, O_SLAB)],
                            scalar1=SHB - b, scalar2=MASK,
                            op0=mybir.AluOpType.logical_shift_left,
                            op1=mybir.AluOpType.bitwise_and,
                        )
                return wt

            wt0 = unpack_slab(0)

            # pw16 cast leads the Pool queue (descgen ~2us), then x
            nc.gpsimd.dma_start(out=pw16_d[:], in_=pw_d[:])

            # ---- x chunks; kt=0 via HWDGE f32 + ACT cast (first data
            # with no SWDGE descriptor-gen latency), kt>=1 via SWDGE ----
            xns = {}
            for kt in range(KT):
                for tt in range(TT):
                    src_ap = x_d[ts(tt, P), ts(kt, 1024)].rearrange(
                        "p (k b) -> p k b", b=8
                    )
                    xn = xn_pool.tile([P, P, 8], BF16, tag="xn16")
                    if kt == 0:
                        x32 = x32_pool.tile([P, P, 8], F32)
                        nc.sync.dma_start(x32[:], src_ap)
                        nc.scalar.activation(out=xn[:], in_=x32[:], func=ACT_COPY)
                    else:
                        nc.gpsimd.dma_start(xn[:], src_ap)
                    xns[kt, tt] = xn

            # XBARs for slab 1 first (its unpack is needed ~20us in),
            # then the rest
            for kt in range(KT):
                nc.sync.dma_start_transpose(
                    byteT[:, kt, ts(1, O_SLAB)],
                    pw16_d[ts(1, O_SLAB), ts(kt, P)],
                )
            for kt in range(KT):
                nc.sync.dma_start_transpose(
                    byteT[:, kt, 2 * O_SLAB:],
                    pw16_d[2 * O_SLAB:, ts(kt, P)],
                )

            wt1 = unpack_slab(1)

            # ---- transpose (bf16) per kt group, then split hi/lo fp8
            # per (kt, tt) so the tail of the cast chain is short; all
            # copies + hi casts on ACT, lo on DVE ----
            for kt in range(KT):
                xtb = xtb_pool.tile([P, 8, T], BF16)
                for tt in range(TT):
                    for b in range(8):
                        ps = ps_tr.tile([P, P], BF16, tag="tr_ps")
                        nc.tensor.transpose(
                            ps[:], xns[kt, tt][:, :, b], ident_bf[:]
                        )
                        dst = xtb[:, b, ts(tt, P)]
                        nc.scalar.activation(out=dst, in_=ps[:], func=ACT_COPY)
                    hi = xT_hi[:, ts(kt, 8), ts(tt, P)]
                    lo = xT_lo[:, ts(kt, 8), ts(tt, P)]
                    src = xtb[:, :, ts(tt, P)]
                    nc.scalar.activation(out=hi, in_=src, func=ACT_COPY)
                    nc.vector.tensor_tensor(
                        out=lo, in0=src, in1=hi, op=mybir.AluOpType.subtract
                    )

            # ---- main o-slab loop (s_col DRs slot in after the first
            # group's DR stream; each group's rank-2 finisher needs s2) ----
            def emit_s_col():
                ps_s = ps_s_pool.tile([P, T], F32)
                for pi, plane in enumerate((xT_hi, xT_lo)):
                    for j in range(NJ):
                        nc.tensor.matmul(
                            ps_s[:], ones2[:], plane[:, 2 * j:2 * j + 2, :],
                            start=(pi == 0 and j == 0),
                            stop=(pi == 1 and j == NJ - 1),
                            perf_mode=DR,
                        )
                nc.vector.tensor_copy(out=s2[0:1, :], in_=ps_s[0:1, :])

            emit_s_col()

            for sl in range(NSLAB):
                if sl == 0:
                    wt = wt0
                elif sl == 1:
                    wt = wt1
                else:
                    wt = unpack_slab(sl)
                wt8 = wt[:].bitcast(F8).rearrange(
                    "p it (n two) -> p it n two", two=2
                )
                for tsub in range(TT):
                    ps = ps_mm.tile([P, O_SLAB], F32)
                    for j in range(NJ):
                        for plane in (xT_hi, xT_lo):
                            nc.tensor.matmul(
                                ps[:],
                                plane[:, 2 * j:2 * j + 2, ts(tsub, P)],
                                wt8[:, 2 * j:2 * j + 2, :, 1],
                                start=(j == 0 and plane is xT_hi),
                                stop=False,
                                perf_mode=DR,
                            )
                    # bias - s_col enters last so the DR stream above can
                    # start before s_col is known
                    nc.tensor.matmul(
                        ps[:], s2[:, ts(tsub, P)], br2[:, ts(sl, O_SLAB)],
                        start=False, stop=True,
                    )
                    y_sb = y_pool.tile([P, O_SLAB], F32)
                    nc.vector.tensor_copy(out=y_sb[:], in_=ps[:])
                    nc.sync.dma_start(
                        y_d[ts(tsub, P), ts(sl, O_SLAB)], y_sb[:]
                    )

    nc.compile()
    return nc


_NC = None


def _get_nc():
    global _NC
    if _NC is None:
        _NC = build()
    return _NC


def run(x, packed_weight, bias, trace=False):
    x = np.ascontiguousarray(np.asarray(x, dtype=np.float32))
    pw = np.ascontiguousarray(np.asarray(packed_weight, dtype=np.uint8))
    bias = np.ascontiguousarray(np.asarray(bias, dtype=np.float32))
    assert x.shape == (B_DIM, S_DIM, I_DIM)
    assert pw.shape == (O_DIM, I_DIM // 8)
    assert bias.shape == (O_DIM,)

    nc = _get_nc()
    xs = x.reshape(T_FULL, I_DIM)
    in_maps = [
        {
            "x": np.ascontiguousarray(xs[c * T_SHARD:(c + 1) * T_SHARD]),
            "pw": pw,
            "bias": bias,
        }
        for c in range(N_CORES)
    ]
    res = run_bass_kernel_spmd(nc, in_maps, list(range(N_CORES)), trace=trace)
    y = np.concatenate(
        [res.results[c][OUT_NAME] for c in range(N_CORES)], axis=0
    )
    return y.reshape(B_DIM, S_DIM, O_DIM), res


def kernel(x, packed_weight, bias):
    y, _ = run(x, packed_weight, bias, trace=False)
    return y


# revision 26
# speedup vs baseline: 1.4826x; 1.0553x over previous
"""BitPackedLinear Trainium2 kernel (8-core SPMD, token-sharded, fp8 DR).

y = x @ W.T + bias, W = unpack_bits(packed_weight) in {-1,+1}, shapes:
  x [2, 2048, 4096] f32, packed_weight [4096, 512] u8, bias [4096] f32.

Sharding: data-parallel over tokens (4096 tokens -> 512/core). Each core
computes y_c = x_c @ W.T + bias for its token shard against the full
weight; the host just concatenates shards.

Device algorithm per core (main GEMM in fp8e4 DoubleRow, 0.5 cyc/row --
2x the bf16 row rate, contracting two 128-i blocks per instruction):
  - x is split exactly as x = hi + lo with hi = fp8(bf16(x)),
    lo = fp8(bf16(x) - hi); W is unpacked to {0, 2.0} and the result
    corrected as y = (hi+lo)@(2B) - rowsum(hi+lo) + bias.  Total split
    error ~1.2e-3 rel (bf16-level).
  - x chunks arrive bf16 (SWDGE cast-DMA; kt=0 via HWDGE f32 + DVE cast
    to dodge the slow Q7 descriptor-gen start), bit-sliced layout.
  - PE transposes x per [128,128] bf16 tile into per-kt groups, then one
    ACT copy casts the group to the fp8 hi plane and one DVE subtract
    produces the fp8 lo plane: xT_hi/xT_lo [128 i-part, 32 it, 512 tok],
    tokens contiguous (the dual-fp8 Ldweights ISA check requires a
    unit-stride stationary m dim; the moving ifmap tolerates stride 2).
  - byteT[k',kt,o] = pw[o,128kt+k'] via u8->u16 cast-DMA bounce through
    DRAM + one XBAR transpose-DMA per kt (no PE/DVE involvement).
  - W unpack stays 1 op/tile: wt16 = (u16(byte) << (14-b)) & 0x4000
    puts fp8 {0, 2.0} in the HIGH byte; the DR matmul reads odd bytes
    ([p, 2, 512] pair-stride 1024, n-stride 2 -- validated on hw).
  - Main DR matmuls per (o-slab, token-tile), for j in 0..16, term in
    (hi, lo): psum += plane[:, 2j:2j+2, t].T (x) wt8[:, 2j:2j+2, :].
  - s_col = rowsum(hi+lo) computed EXACTLY by 32 DR matmuls against an
    all-ones stationary -> psum row -> SBUF f32r row.
  - bias + (-s_col) enter each psum via ONE rank-2 f32r matmul
    (k=0: s_col x -1, k=1: ones x bias) with start=True; epilogue is a
    plain psum->SBUF copy + DMA.
"""
import sys

sys.path.insert(0, "/opt/trn_rl_repo")
from contextlib import ExitStack

import numpy as np

import concourse.tile as tile
from concourse import bacc, mybir
from concourse.bass import ts
from concourse.bass_utils import run_bass_kernel_spmd
from concourse.masks import make_identity

F32 = mybir.dt.float32
F32R = mybir.dt.float32r
BF16 = mybir.dt.bfloat16
F8 = mybir.dt.float8e4
U8 = mybir.dt.uint8
U16 = mybir.dt.uint16
P = 128
DR = mybir.MatmulPerfMode.DoubleRow
ACT_COPY = mybir.ActivationFunctionType.Copy

N_CORES = 8
B_DIM, S_DIM, I_DIM, O_DIM = 2, 2048, 4096, 4096
T_FULL = B_DIM * S_DIM          # 4096 tokens
T_SHARD = T_FULL // N_CORES     # 512 tokens per core
OUT_NAME = "y"
OUT_SHAPE = (T_SHARD, O_DIM)


def build(T=T_SHARD, I=I_DIM, O=O_DIM, O_SLAB=512, n_cores=N_CORES,
          byte_mode="dmat"):
    assert I % 1024 == 0 and T % P == 0 and O % P == 0 and O % O_SLAB == 0
    KT = I // 1024          # 128-byte groups along i (4)
    IT = KT * 8             # bit-sliced i-tiles (32)
    NJ = IT // 2            # DR i-tile pairs (16)
    TT = T // P             # token tiles (4)
    K = I // 8              # packed bytes per weight row (512)
    NSLAB = O // O_SLAB
    OSL_T = O_SLAB // P
    SHB, MASK = 14, 0x4000  # unpack: fp8 {0,2.0} pattern in the HIGH byte

    nc = bacc.Bacc("TRN2", target_bir_lowering=False, debug=False,
                   num_devices=n_cores)
    x_d = nc.dram_tensor("x", [T, I], F32, kind="ExternalInput").ap()
    pw_d = nc.dram_tensor("pw", [O, K], U8, kind="ExternalInput").ap()
    bias_d = nc.dram_tensor("bias", [O], F32, kind="ExternalInput").ap()
    y_d = nc.dram_tensor(OUT_NAME, [T, O], F32, kind="ExternalOutput").ap()
    pw16_d = nc.dram_tensor("pw16", [O, K], U16).ap()

    with tile.TileContext(nc) as tc:
        with ExitStack() as ctx:
            const = ctx.enter_context(tc.tile_pool(name="const", bufs=1))
            persist = ctx.enter_context(tc.tile_pool(name="persist", bufs=1))
            stage = ctx.enter_context(tc.tile_pool(name="stage", bufs=1))

            ident_bf = const.tile([P, P], BF16)
            make_identity(nc, ident_bf[:])
            ones2 = const.tile([P, 2, P], F8)
            nc.vector.memset(ones2[:], 1.0)

            # rank-2 psum-init operands; engine-written rows sit at
            # partition 0 (engine APs must start on a quarter boundary),
            # partition-1 rows are DMA-filled.
            # ps = s2[0,t]*br2[0,o] + s2[1,t]*br2[1,o]
            #    = s_col[t]*(-1)    + 1*bias[o]
            s2 = const.tile([2, T], F32R)    # row0 = s_col, row1 = ones
            br2 = const.tile([2, O], F32R)   # row0 = -ones, row1 = bias

            byteT = persist.tile([P, KT, O], U16)
            xT_hi = persist.tile([P, IT, T], F8)
            xT_lo = persist.tile([P, IT, T], F8)

            x32_pool = ctx.enter_context(tc.tile_pool(name="x32", bufs=2))
            xn_pool = ctx.enter_context(tc.tile_pool(name="xnat", bufs=10))
            pk_pool = ctx.enter_context(tc.tile_pool(name="pk", bufs=2))
            pkbf_pool = ctx.enter_context(tc.tile_pool(name="pkbf", bufs=2))
            wt_pool = ctx.enter_context(tc.tile_pool(name="wt", bufs=2))
            y_pool = ctx.enter_context(tc.tile_pool(name="ysb", bufs=3))
            ps_tr = ctx.enter_context(
                tc.tile_pool(name="ps_tr", bufs=2, space="PSUM")
            )
            ps_mm = ctx.enter_context(
                tc.tile_pool(name="ps_mm", bufs=5, space="PSUM")
            )
            ps_s_pool = ctx.enter_context(
                tc.tile_pool(name="ps_s", bufs=1, space="PSUM")
            )


            def emit_bias_stage():
                nc.sync.dma_start(
                    br2[1:2, :],
                    bias_d.bitcast(F32R).rearrange("(b o) -> b o", b=1),
                )
                cst = stage.tile([1, T], F32)
                nc.vector.memset(cst[:], -1.0)
                nc.vector.tensor_copy(
                    out=br2[0:1, :],
                    in_=cst[:, :1].broadcast_to([1, O]),
                )
                ones_row = stage.tile([1, T], F32R)
                nc.vector.memset(cst[:], 1.0)
                nc.vector.tensor_copy(out=ones_row[:], in_=cst[:])
                nc.sync.dma_start(s2[1:2, :], ones_row[:])

            def byte_slab(sl):
                """Fill byteT[:, :, sl*O_SLAB:(sl+1)*O_SLAB] via PE."""
                pw_ap = pw_d.rearrange("(ot p) k -> p ot k", p=P)
                pk = pk_pool.tile([P, OSL_T, K], U8)
                nc.sync.dma_start(pk[:], pw_ap[:, ts(sl, OSL_T), :])
                for otl in range(OSL_T):
                    ot = sl * OSL_T + otl
                    pkbf = pkbf_pool.tile([P, K], BF16)
                    nc.any.tensor_copy(out=pkbf[:], in_=pk[:, otl, :])
                    for kt in range(KT):
                        ps = ps_tr.tile([P, P], BF16, tag="tr_ps")
                        nc.tensor.transpose(ps[:], pkbf[:, ts(kt, P)], ident_bf[:])
                        nc.any.tensor_copy(out=byteT[:, kt, ts(ot, P)], in_=ps[:])

            # ---- byte path: slab 0 via the PE path (ready in a few us,
            # no DMA-engine contention); slabs 1-7 via pw16 bounce + XBAR
            # transpose-DMAs emitted after the x chunks ----
            byte_slab(0)

            emit_bias_stage()

            # PE warm-up: ~24 back-to-back identity transposes ramp the
            # p-state to full clock while the first DMAs land.
            ps_warm = ps_tr.tile([P, T], BF16, tag="tr_ps")
            for i in range(24):
                nc.tensor.transpose(
                    ps_warm[:, ts(i % 4, P)], ident_bf[:], ident_bf[:]
                )

            # slab-0/1 unpacks lead the DVE queue
            def unpack_slab(sl):
                wt = wt_pool.tile([P, IT, O_SLAB], U16)
                for kt in range(KT):
                    for b in range(8):
                        it = kt * 8 + b
                        nc.vector.tensor_scalar(
                            out=wt[:, it, :],
                            in0=byteT[:, kt, ts(sl, O_SLAB)],
                            scalar1=SHB - b, scalar2=MASK,
                            op0=mybir.AluOpType.logical_shift_left,
                            op1=mybir.AluOpType.bitwise_and,
                        )
                return wt

            wt0 = unpack_slab(0)

            # ---- x chunks; kt=0 via HWDGE f32 + ACT cast (first data
            # with no SWDGE descriptor-gen latency), kt>=1 via SWDGE ----
            xns = {}
            for kt in range(KT):
                for tt in range(TT):
                    src_ap = x_d[ts(tt, P), ts(kt, 1024)].rearrange(
                        "p (k b) -> p k b", b=8
                    )
                    xn = xn_pool.tile([P, P, 8], BF16, tag="xn16")
                    if kt == 0:
                        x32 = x32_pool.tile([P, P, 8], F32)
                        nc.sync.dma_start(x32[:], src_ap)
                        nc.scalar.activation(out=xn[:], in_=x32[:], func=ACT_COPY)
                    else:
                        nc.gpsimd.dma_start(xn[:], src_ap)
                    xns[kt, tt] = xn

            # pw16 bounce AFTER the x chunks (an 11us DMA transfer must
            # not block them), split in two so slab-1 bytes arrive first
            nc.gpsimd.dma_start(
                out=pw16_d[O_SLAB:3 * O_SLAB, :], in_=pw_d[O_SLAB:3 * O_SLAB, :]
            )
            for kt in range(KT):
                nc.sync.dma_start_transpose(
                    byteT[:, kt, O_SLAB:3 * O_SLAB],
                    pw16_d[O_SLAB:3 * O_SLAB, ts(kt, P)],
                )
            nc.gpsimd.dma_start(
                out=pw16_d[3 * O_SLAB:, :], in_=pw_d[3 * O_SLAB:, :]
            )
            for kt in range(KT):
                nc.sync.dma_start_transpose(
                    byteT[:, kt, 3 * O_SLAB:],
                    pw16_d[3 * O_SLAB:, ts(kt, P)],
                )

            wt1 = unpack_slab(1)

            # ---- transposes, 4 token-tiles batched into one [128, 512]
            # PSUM tile per i-tile; hi/lo casts read PSUM directly ----
            for kt in range(KT):
                for b in range(8):
                    it = kt * 8 + b
                    ps = ps_tr.tile([P, T], BF16, tag="tr_ps")
                    for tt in range(TT):
                        nc.tensor.transpose(
                            ps[:, ts(tt, P)], xns[kt, tt][:, :, b], ident_bf[:]
                        )
                    hi = xT_hi[:, it, :]
                    lo = xT_lo[:, it, :]
                    nc.scalar.activation(out=hi, in_=ps[:], func=ACT_COPY)
                    nc.vector.tensor_tensor(
                        out=lo, in0=ps[:], in1=hi, op=mybir.AluOpType.subtract
                    )

            # ---- main o-slab loop (s_col DRs slot in after the first
            # group's DR stream; each group's rank-2 finisher needs s2) ----
            def emit_s_col():
                ps_s = ps_s_pool.tile([P, T], F32)
                for pi, plane in enumerate((xT_hi, xT_lo)):
                    for j in range(NJ):
                        nc.tensor.matmul(
                            ps_s[:], ones2[:], plane[:, 2 * j:2 * j + 2, :],
                            start=(pi == 0 and j == 0),
                            stop=(pi == 1 and j == NJ - 1),
                            perf_mode=DR,
                        )
                nc.vector.tensor_copy(out=s2[0:1, :], in_=ps_s[0:1, :])

            emit_s_col()

            for sl in range(NSLAB):
                if sl == 0:
                    wt = wt0
                elif sl == 1:
                    wt = wt1
                else:
                    wt = unpack_slab(sl)
                wt8 = wt[:].bitcast(F8).rearrange(
                    "p it (n two) -> p it n two", two=2
                )
                for tsub in range(TT):
                    ps = ps_mm.tile([P, O_SLAB], F32)
                    for j in range(NJ):
                        for plane in (xT_hi, xT_lo):
                            nc.tensor.matmul(
                                ps[:],
                                plane[:, 2 * j:2 * j + 2, ts(tsub, P)],
                                wt8[:, 2 * j:2 * j + 2, :, 1],
                                start=(j == 0 and plane is xT_hi),
                                stop=False,
                                perf_mode=DR,
                            )
                    # bias - s_col enters last so the DR stream above can
                    # start before s_col is known
                    nc.tensor.matmul(
                        ps[:], s2[:, ts(tsub, P)], br2[:, ts(sl, O_SLAB)],
                        start=False, stop=True,
                    )
                    y_sb = y_pool.tile([P, O_SLAB], F32)
                    nc.scalar.activation(out=y_sb[:], in_=ps[:], func=ACT_COPY)
                    nc.sync.dma_start(
                        y_d[ts(tsub, P), ts(sl, O_SLAB)], y_sb[:]
                    )

    nc.compile()
    return nc


_NC = None


def _get_nc():
    global _NC
    if _NC is None:
        _NC = build()
    return _NC


def run(x, packed_weight, bias, trace=False):
    x = np.ascontiguousarray(np.asarray(x, dtype=np.float32))
    pw = np.ascontiguousarray(np.asarray(packed_weight, dtype=np.uint8))
    bias = np.ascontiguousarray(np.asarray(bias, dtype=np.float32))
    assert x.shape == (B_DIM, S_DIM, I_DIM)
    assert pw.shape == (O_DIM, I_DIM // 8)
    assert bias.shape == (O_DIM,)

    nc = _get_nc()
    xs = x.reshape(T_FULL, I_DIM)
    in_maps = [
        {
            "x": np.ascontiguousarray(xs[c * T_SHARD:(c + 1) * T_SHARD]),
            "pw": pw,
            "bias": bias,
        }
        for c in range(N_CORES)
    ]
    res = run_bass_kernel_spmd(nc, in_maps, list(range(N_CORES)), trace=trace)
    y = np.concatenate(
        [res.results[c][OUT_NAME] for c in range(N_CORES)], axis=0
    )
    return y.reshape(B_DIM, S_DIM, O_DIM), res


def kernel(x, packed_weight, bias):
    y, _ = run(x, packed_weight, bias, trace=False)
    return y


# revision 28
# speedup vs baseline: 1.5170x; 1.0232x over previous
"""BitPackedLinear Trainium2 kernel (8-core SPMD, token-sharded, fp8 DR).

y = x @ W.T + bias, W = unpack_bits(packed_weight) in {-1,+1}, shapes:
  x [2, 2048, 4096] f32, packed_weight [4096, 512] u8, bias [4096] f32.

Sharding: data-parallel over tokens (4096 tokens -> 512/core). Each core
computes y_c = x_c @ W.T + bias for its token shard against the full
weight; the host just concatenates shards.

Device algorithm per core (main GEMM in fp8e4 DoubleRow, 0.5 cyc/row --
2x the bf16 row rate, contracting two 128-i blocks per instruction):
  - x is split exactly as x = hi + lo with hi = fp8(bf16(x)),
    lo = fp8(bf16(x) - hi); W is unpacked to {0, 2.0} and the result
    corrected as y = (hi+lo)@(2B) - rowsum(hi+lo) + bias.  Total split
    error ~1.2e-3 rel (bf16-level).
  - x chunks arrive bf16 (SWDGE cast-DMA; kt=0 via HWDGE f32 + DVE cast
    to dodge the slow Q7 descriptor-gen start), bit-sliced layout.
  - PE transposes x per [128,128] bf16 tile into per-kt groups, then one
    ACT copy casts the group to the fp8 hi plane and one DVE subtract
    produces the fp8 lo plane: xT_hi/xT_lo [128 i-part, 32 it, 512 tok],
    tokens contiguous (the dual-fp8 Ldweights ISA check requires a
    unit-stride stationary m dim; the moving ifmap tolerates stride 2).
  - byteT[k',kt,o] = pw[o,128kt+k'] via u8->u16 cast-DMA bounce through
    DRAM + one XBAR transpose-DMA per kt (no PE/DVE involvement).
  - W unpack stays 1 op/tile: wt16 = (u16(byte) << (14-b)) & 0x4000
    puts fp8 {0, 2.0} in the HIGH byte; the DR matmul reads odd bytes
    ([p, 2, 512] pair-stride 1024, n-stride 2 -- validated on hw).
  - Main DR matmuls per (o-slab, token-tile), for j in 0..16, term in
    (hi, lo): psum += plane[:, 2j:2j+2, t].T (x) wt8[:, 2j:2j+2, :].
  - s_col = rowsum(hi+lo) computed EXACTLY by 32 DR matmuls against an
    all-ones stationary -> psum row -> SBUF f32r row.
  - bias + (-s_col) enter each psum via ONE rank-2 f32r matmul
    (k=0: s_col x -1, k=1: ones x bias) with start=True; epilogue is a
    plain psum->SBUF copy + DMA.
"""
import sys

sys.path.insert(0, "/opt/trn_rl_repo")
from contextlib import ExitStack

import numpy as np

import concourse.tile as tile
from concourse import bacc, mybir
from concourse.bass import ts
from concourse.bass_utils import run_bass_kernel_spmd
from concourse.masks import make_identity

F32 = mybir.dt.float32
F32R = mybir.dt.float32r
BF16 = mybir.dt.bfloat16
F8 = mybir.dt.float8e4
U8 = mybir.dt.uint8
U16 = mybir.dt.uint16
P = 128
DR = mybir.MatmulPerfMode.DoubleRow
ACT_COPY = mybir.ActivationFunctionType.Copy

N_CORES = 8
B_DIM, S_DIM, I_DIM, O_DIM = 2, 2048, 4096, 4096
T_FULL = B_DIM * S_DIM          # 4096 tokens
T_SHARD = T_FULL // N_CORES     # 512 tokens per core
OUT_NAME = "y"
OUT_SHAPE = (T_SHARD, O_DIM)


def build(T=T_SHARD, I=I_DIM, O=O_DIM, O_SLAB=512, n_cores=N_CORES,
          byte_mode="dmat"):
    assert I % 1024 == 0 and T % P == 0 and O % P == 0 and O % O_SLAB == 0
    KT = I // 1024          # 128-byte groups along i (4)
    IT = KT * 8             # bit-sliced i-tiles (32)
    NJ = IT // 2            # DR i-tile pairs (16)
    TT = T // P             # token tiles (4)
    K = I // 8              # packed bytes per weight row (512)
    NSLAB = O // O_SLAB
    OSL_T = O_SLAB // P
    SHB, MASK = 14, 0x4000  # unpack: fp8 {0,2.0} pattern in the HIGH byte

    nc = bacc.Bacc("TRN2", target_bir_lowering=False, debug=False,
                   num_devices=n_cores)
    x_d = nc.dram_tensor("x", [T, I], F32, kind="ExternalInput").ap()
    pw_d = nc.dram_tensor("pw", [O, K], U8, kind="ExternalInput").ap()
    bias_d = nc.dram_tensor("bias", [O], F32, kind="ExternalInput").ap()
    y_d = nc.dram_tensor(OUT_NAME, [T, O], F32, kind="ExternalOutput").ap()
    pw16_d = nc.dram_tensor("pw16", [O, K], U16).ap()

    with tile.TileContext(nc) as tc:
        with ExitStack() as ctx:
            const = ctx.enter_context(tc.tile_pool(name="const", bufs=1))
            persist = ctx.enter_context(tc.tile_pool(name="persist", bufs=1))
            stage = ctx.enter_context(tc.tile_pool(name="stage", bufs=1))

            ident_bf = const.tile([P, P], BF16)
            make_identity(nc, ident_bf[:])
            ones2 = const.tile([P, 2, P], F8)
            nc.vector.memset(ones2[:], 1.0)

            # rank-2 psum-init operands; engine-written rows sit at
            # partition 0 (engine APs must start on a quarter boundary),
            # partition-1 rows are DMA-filled.
            # ps = s2[0,t]*br2[0,o] + s2[1,t]*br2[1,o]
            #    = s_col[t]*(-1)    + 1*bias[o]
            s2 = const.tile([2, T], F32R)    # row0 = s_col, row1 = ones
            br2 = const.tile([2, O], F32R)   # row0 = -ones, row1 = bias

            byteT = persist.tile([P, KT, O], U16)
            xT_hi = persist.tile([P, IT, T], F8)
            xT_lo = persist.tile([P, IT, T], F8)

            x32_pool = ctx.enter_context(tc.tile_pool(name="x32", bufs=2))
            xn_pool = ctx.enter_context(tc.tile_pool(name="xnat", bufs=10))
            pk_pool = ctx.enter_context(tc.tile_pool(name="pk", bufs=2))
            pkbf_pool = ctx.enter_context(tc.tile_pool(name="pkbf", bufs=2))
            wt_pool = ctx.enter_context(tc.tile_pool(name="wt", bufs=2))
            y_pool = ctx.enter_context(tc.tile_pool(name="ysb", bufs=3))
            ps_tr = ctx.enter_context(
                tc.tile_pool(name="ps_tr", bufs=2, space="PSUM")
            )
            ps_mm = ctx.enter_context(
                tc.tile_pool(name="ps_mm", bufs=5, space="PSUM")
            )
            ps_s_pool = ctx.enter_context(
                tc.tile_pool(name="ps_s", bufs=1, space="PSUM")
            )


            def emit_bias_stage():
                nc.sync.dma_start(
                    br2[1:2, :],
                    bias_d.bitcast(F32R).rearrange("(b o) -> b o", b=1),
                )
                cst = stage.tile([1, T], F32)
                nc.vector.memset(cst[:], -1.0)
                nc.vector.tensor_copy(
                    out=br2[0:1, :],
                    in_=cst[:, :1].broadcast_to([1, O]),
                )
                ones_row = stage.tile([1, T], F32R)
                nc.vector.memset(cst[:], 1.0)
                nc.vector.tensor_copy(out=ones_row[:], in_=cst[:])
                nc.sync.dma_start(s2[1:2, :], ones_row[:])

            def byte_slab(sl):
                """Fill byteT[:, :, sl*O_SLAB:(sl+1)*O_SLAB] via PE."""
                pw_ap = pw_d.rearrange("(ot p) k -> p ot k", p=P)
                pk = pk_pool.tile([P, OSL_T, K], U8)
                nc.sync.dma_start(pk[:], pw_ap[:, ts(sl, OSL_T), :])
                for otl in range(OSL_T):
                    ot = sl * OSL_T + otl
                    pkbf = pkbf_pool.tile([P, K], BF16)
                    nc.any.tensor_copy(out=pkbf[:], in_=pk[:, otl, :])
                    for kt in range(KT):
                        ps = ps_tr.tile([P, P], BF16, tag="tr_ps")
                        nc.tensor.transpose(ps[:], pkbf[:, ts(kt, P)], ident_bf[:])
                        nc.any.tensor_copy(out=byteT[:, kt, ts(ot, P)], in_=ps[:])

            emit_bias_stage()

            # PE warm-up: back-to-back identity transposes ramp the
            # p-state to full clock while the first DMAs land.
            ps_warm = ps_tr.tile([P, 8, P], BF16, tag="tr_ps")
            for i in range(24):
                nc.tensor.transpose(
                    ps_warm[:, i % 8, :], ident_bf[:], ident_bf[:]
                )

            # slab-0/1 unpacks lead the DVE queue
            def unpack_slab(sl):
                wt = wt_pool.tile([P, IT, O_SLAB], U16)
                for kt in range(KT):
                    for b in range(8):
                        it = kt * 8 + b
                        nc.vector.tensor_scalar(
                            out=wt[:, it, :],
                            in0=byteT[:, kt, ts(sl, O_SLAB)],
                            scalar1=SHB - b, scalar2=MASK,
                            op0=mybir.AluOpType.logical_shift_left,
                            op1=mybir.AluOpType.bitwise_and,
                        )
                return wt

            # ---- x chunks; kt=0 via HWDGE f32 + ACT cast (first data
            # with no SWDGE descriptor-gen latency), kt>=1 via SWDGE ----
            xns = {}
            for kt in range(KT):
                for tt in range(TT):
                    src_ap = x_d[ts(tt, P), ts(kt, 1024)].rearrange(
                        "p (k b) -> p k b", b=8
                    )
                    xn = xn_pool.tile([P, P, 8], BF16, tag="xn16")
                    if kt == 0:
                        x32 = x32_pool.tile([P, P, 8], F32)
                        nc.sync.dma_start(x32[:], src_ap)
                        nc.scalar.activation(out=xn[:], in_=x32[:], func=ACT_COPY)
                    else:
                        nc.gpsimd.dma_start(xn[:], src_ap)
                    xns[kt, tt] = xn

            # slab-0 bytes via the PE path (emitted after x so its pk DMA
            # doesn't delay the first x chunks)
            byte_slab(0)
            wt0 = unpack_slab(0)

            # pw16 bounce AFTER the x chunks (an 11us DMA transfer must
            # not block them), split in two so slab-1 bytes arrive first
            nc.gpsimd.dma_start(
                out=pw16_d[O_SLAB:3 * O_SLAB, :], in_=pw_d[O_SLAB:3 * O_SLAB, :]
            )
            for kt in range(KT):
                nc.sync.dma_start_transpose(
                    byteT[:, kt, O_SLAB:3 * O_SLAB],
                    pw16_d[O_SLAB:3 * O_SLAB, ts(kt, P)],
                )
            nc.gpsimd.dma_start(
                out=pw16_d[3 * O_SLAB:, :], in_=pw_d[3 * O_SLAB:, :]
            )
            for kt in range(KT):
                nc.sync.dma_start_transpose(
                    byteT[:, kt, 3 * O_SLAB:],
                    pw16_d[3 * O_SLAB:, ts(kt, P)],
                )

            wt1 = unpack_slab(1)

            # ---- transposes, 8 bit-planes of one chunk batched into one
            # [128, 1024] PSUM bank; hi/lo casts read PSUM directly.
            # Each batch depends on a single x chunk. ----
            for kt in range(KT):
                for tt in range(TT):
                    ps = ps_tr.tile([P, 8, P], BF16, tag="tr_ps")
                    for b in range(8):
                        nc.tensor.transpose(
                            ps[:, b, :], xns[kt, tt][:, :, b], ident_bf[:]
                        )
                    hi = xT_hi[:, ts(kt, 8), ts(tt, P)]
                    lo = xT_lo[:, ts(kt, 8), ts(tt, P)]
                    nc.scalar.activation(out=hi, in_=ps[:], func=ACT_COPY)
                    nc.vector.tensor_tensor(
                        out=lo, in0=ps[:], in1=hi, op=mybir.AluOpType.subtract
                    )

            # ---- main o-slab loop (s_col DRs slot in after the first
            # group's DR stream; each group's rank-2 finisher needs s2) ----
            def emit_s_col():
                ps_s = ps_s_pool.tile([P, T], F32)
                for pi, plane in enumerate((xT_hi, xT_lo)):
                    for j in range(NJ):
                        nc.tensor.matmul(
                            ps_s[:], ones2[:], plane[:, 2 * j:2 * j + 2, :],
                            start=(pi == 0 and j == 0),
                            stop=(pi == 1 and j == NJ - 1),
                            perf_mode=DR,
                        )
                nc.vector.tensor_copy(out=s2[0:1, :], in_=ps_s[0:1, :])

            emit_s_col()

            for sl in range(NSLAB):
                if sl == 0:
                    wt = wt0
                elif sl == 1:
                    wt = wt1
                else:
                    wt = unpack_slab(sl)
                wt8 = wt[:].bitcast(F8).rearrange(
                    "p it (n two) -> p it n two", two=2
                )
                for tsub in range(TT):
                    ps = ps_mm.tile([P, O_SLAB], F32)
                    for j in range(NJ):
                        for plane in (xT_hi, xT_lo):
                            nc.tensor.matmul(
                                ps[:],
                                plane[:, 2 * j:2 * j + 2, ts(tsub, P)],
                                wt8[:, 2 * j:2 * j + 2, :, 1],
                                start=(j == 0 and plane is xT_hi),
                                stop=False,
                                perf_mode=DR,
                            )
                    # bias - s_col enters last so the DR stream above can
                    # start before s_col is known
                    nc.tensor.matmul(
                        ps[:], s2[:, ts(tsub, P)], br2[:, ts(sl, O_SLAB)],
                        start=False, stop=True,
                    )
                    y_sb = y_pool.tile([P, O_SLAB], F32)
                    nc.scalar.activation(out=y_sb[:], in_=ps[:], func=ACT_COPY)
                    nc.sync.dma_start(
                        y_d[ts(tsub, P), ts(sl, O_SLAB)], y_sb[:]
                    )

    nc.compile()
    return nc


_NC = None


def _get_nc():
    global _NC
    if _NC is None:
        _NC = build()
    return _NC


def run(x, packed_weight, bias, trace=False):
    x = np.ascontiguousarray(np.asarray(x, dtype=np.float32))
    pw = np.ascontiguousarray(np.asarray(packed_weight, dtype=np.uint8))
    bias = np.ascontiguousarray(np.asarray(bias, dtype=np.float32))
    assert x.shape == (B_DIM, S_DIM, I_DIM)
    assert pw.shape == (O_DIM, I_DIM // 8)
    assert bias.shape == (O_DIM,)

    nc = _get_nc()
    xs = x.reshape(T_FULL, I_DIM)
    in_maps = [
        {
            "x": np.ascontiguousarray(xs[c * T_SHARD:(c + 1) * T_SHARD]),
            "pw": pw,
            "bias": bias,
        }
        for c in range(N_CORES)
    ]
    res = run_bass_kernel_spmd(nc, in_maps, list(range(N_CORES)), trace=trace)
    y = np.concatenate(
        [res.results[c][OUT_NAME] for c in range(N_CORES)], axis=0
    )
    return y.reshape(B_DIM, S_DIM, O_DIM), res


def kernel(x, packed_weight, bias):
    y, _ = run(x, packed_weight, bias, trace=False)
    return y


# revision 32
# speedup vs baseline: 1.5827x; 1.0433x over previous
"""BitPackedLinear Trainium2 kernel (8-core SPMD, token-sharded, fp8 DR).

y = x @ W.T + bias, W = unpack_bits(packed_weight) in {-1,+1}, shapes:
  x [2, 2048, 4096] f32, packed_weight [4096, 512] u8, bias [4096] f32.

Sharding: data-parallel over tokens (4096 tokens -> 512/core). Each core
computes y_c = x_c @ W.T + bias for its token shard against the full
weight; the host just concatenates shards.

Device algorithm per core (main GEMM in fp8e4 DoubleRow, 0.5 cyc/row --
2x the bf16 row rate, contracting two 128-i blocks per instruction):
  - x is split exactly as x = hi + lo with hi = fp8(bf16(x)),
    lo = fp8(bf16(x) - hi); W is unpacked to {0, 2.0} and the result
    corrected as y = (hi+lo)@(2B) - rowsum(hi+lo) + bias.  Total split
    error ~1.2e-3 rel (bf16-level).
  - x chunks arrive bf16 (SWDGE cast-DMA; kt=0 via HWDGE f32 + DVE cast
    to dodge the slow Q7 descriptor-gen start), bit-sliced layout.
  - PE transposes x per [128,128] bf16 tile into per-kt groups, then one
    ACT copy casts the group to the fp8 hi plane and one DVE subtract
    produces the fp8 lo plane: xT_hi/xT_lo [128 i-part, 32 it, 512 tok],
    tokens contiguous (the dual-fp8 Ldweights ISA check requires a
    unit-stride stationary m dim; the moving ifmap tolerates stride 2).
  - byteT[k',kt,o] = pw[o,128kt+k'] via u8->u16 cast-DMA bounce through
    DRAM + one XBAR transpose-DMA per kt (no PE/DVE involvement).
  - W unpack stays 1 op/tile: wt16 = (u16(byte) << (14-b)) & 0x4000
    puts fp8 {0, 2.0} in the HIGH byte; the DR matmul reads odd bytes
    ([p, 2, 512] pair-stride 1024, n-stride 2 -- validated on hw).
  - Main DR matmuls per (o-slab, token-tile), for j in 0..16, term in
    (hi, lo): psum += plane[:, 2j:2j+2, t].T (x) wt8[:, 2j:2j+2, :].
  - s_col = rowsum(hi+lo) computed EXACTLY by 32 DR matmuls against an
    all-ones stationary -> psum row -> SBUF f32r row.
  - bias + (-s_col) enter each psum via ONE rank-2 f32r matmul
    (k=0: s_col x -1, k=1: ones x bias) with start=True; epilogue is a
    plain psum->SBUF copy + DMA.
"""
import sys

sys.path.insert(0, "/opt/trn_rl_repo")
from contextlib import ExitStack

import numpy as np

import concourse.tile as tile
from concourse import bacc, mybir
from concourse.bass import ts
from concourse.bass_utils import run_bass_kernel_spmd
from concourse.masks import make_identity

F32 = mybir.dt.float32
F32R = mybir.dt.float32r
BF16 = mybir.dt.bfloat16
F8 = mybir.dt.float8e4
U8 = mybir.dt.uint8
U16 = mybir.dt.uint16
P = 128
DR = mybir.MatmulPerfMode.DoubleRow
ACT_COPY = mybir.ActivationFunctionType.Copy

N_CORES = 8
B_DIM, S_DIM, I_DIM, O_DIM = 2, 2048, 4096, 4096
T_FULL = B_DIM * S_DIM          # 4096 tokens
T_SHARD = T_FULL // N_CORES     # 512 tokens per core
OUT_NAME = "y"
OUT_SHAPE = (T_SHARD, O_DIM)


def build(T=T_SHARD, I=I_DIM, O=O_DIM, O_SLAB=512, n_cores=N_CORES,
          byte_mode="dmat"):
    assert I % 1024 == 0 and T % P == 0 and O % P == 0 and O % O_SLAB == 0
    KT = I // 1024          # 128-byte groups along i (4)
    IT = KT * 8             # bit-sliced i-tiles (32)
    NJ = IT // 2            # DR i-tile pairs (16)
    TT = T // P             # token tiles (4)
    K = I // 8              # packed bytes per weight row (512)
    NSLAB = O // O_SLAB
    OSL_T = O_SLAB // P
    SHB, MASK = 14, 0x4000  # unpack: fp8 {0,2.0} pattern in the HIGH byte

    nc = bacc.Bacc("TRN2", target_bir_lowering=False, debug=False,
                   num_devices=n_cores)
    x_d = nc.dram_tensor("x", [T, I], F32, kind="ExternalInput").ap()
    pw_d = nc.dram_tensor("pw", [O, K], U8, kind="ExternalInput").ap()
    bias_d = nc.dram_tensor("bias", [O], F32, kind="ExternalInput").ap()
    y_d = nc.dram_tensor(OUT_NAME, [T, O], F32, kind="ExternalOutput").ap()
    pw16_d = nc.dram_tensor("pw16", [O, K], U16).ap()

    with tile.TileContext(nc) as tc:
        with ExitStack() as ctx:
            const = ctx.enter_context(tc.tile_pool(name="const", bufs=1))
            persist = ctx.enter_context(tc.tile_pool(name="persist", bufs=1))
            stage = ctx.enter_context(tc.tile_pool(name="stage", bufs=1))

            ident_bf = const.tile([P, P], BF16)
            make_identity(nc, ident_bf[:])
            ones2 = const.tile([P, 2, P], F8)
            nc.vector.memset(ones2[:], 1.0)

            # rank-2 psum-init operands; engine-written rows sit at
            # partition 0 (engine APs must start on a quarter boundary),
            # partition-1 rows are DMA-filled.
            # ps = s2[0,t]*br2[0,o] + s2[1,t]*br2[1,o]
            #    = s_col[t]*(-1)    + 1*bias[o]
            s2 = const.tile([2, T], F32R)    # row0 = s_col, row1 = ones
            br2 = const.tile([2, O], F32R)   # row0 = -ones, row1 = bias

            byteT = persist.tile([P, KT, O], U16)
            xT_hi = persist.tile([P, IT, T], F8)
            xT_lo = persist.tile([P, IT, T], F8)

            x32_pool = ctx.enter_context(tc.tile_pool(name="x32", bufs=2))
            xn_pool = ctx.enter_context(tc.tile_pool(name="xnat", bufs=10))
            pk_pool = ctx.enter_context(tc.tile_pool(name="pk", bufs=2))
            pkbf_pool = ctx.enter_context(tc.tile_pool(name="pkbf", bufs=2))
            wt_pool = ctx.enter_context(tc.tile_pool(name="wt", bufs=2))
            y_pool = ctx.enter_context(tc.tile_pool(name="ysb", bufs=3))
            ps_tr = ctx.enter_context(
                tc.tile_pool(name="ps_tr", bufs=3, space="PSUM")
            )
            ps_mm = ctx.enter_context(
                tc.tile_pool(name="ps_mm", bufs=5, space="PSUM")
            )


            def emit_bias_stage():
                nc.sync.dma_start(
                    br2[1:2, :],
                    bias_d.bitcast(F32R).rearrange("(b o) -> b o", b=1),
                )
                cst = stage.tile([1, T], F32)
                nc.vector.memset(cst[:], -1.0)
                nc.vector.tensor_copy(
                    out=br2[0:1, :],
                    in_=cst[:, :1].broadcast_to([1, O]),
                )
                ones_row = stage.tile([1, T], F32R)
                nc.vector.memset(cst[:], 1.0)
                nc.vector.tensor_copy(out=ones_row[:], in_=cst[:])
                nc.sync.dma_start(s2[1:2, :], ones_row[:])

            def byte_slab(sl):
                """Fill byteT[:, :, sl*O_SLAB:(sl+1)*O_SLAB] via PE."""
                pw_ap = pw_d.rearrange("(ot p) k -> p ot k", p=P)
                pk = pk_pool.tile([P, OSL_T, K], U8)
                nc.sync.dma_start(pk[:], pw_ap[:, ts(sl, OSL_T), :])
                for otl in range(OSL_T):
                    ot = sl * OSL_T + otl
                    pkbf = pkbf_pool.tile([P, K], BF16)
                    nc.any.tensor_copy(out=pkbf[:], in_=pk[:, otl, :])
                    for kt in range(KT):
                        ps = ps_tr.tile([P, P], BF16, tag="tr_ps")
                        nc.tensor.transpose(ps[:], pkbf[:, ts(kt, P)], ident_bf[:])
                        nc.any.tensor_copy(out=byteT[:, kt, ts(ot, P)], in_=ps[:])

            emit_bias_stage()

            # PE warm-up: back-to-back identity transposes ramp the
            # p-state to full clock while the first DMAs land.
            ps_warm = ps_tr.tile([P, 8, P], BF16, tag="tr_ps")
            for i in range(24):
                nc.tensor.transpose(
                    ps_warm[:, i % 8, :], ident_bf[:], ident_bf[:]
                )

            # slab-0/1 unpacks lead the DVE queue
            def unpack_slab(sl):
                wt = wt_pool.tile([P, IT, O_SLAB], U16, tag="wt")
                for kt in range(KT):
                    for b in range(8):
                        it = kt * 8 + b
                        nc.vector.tensor_scalar(
                            out=wt[:, it, :],
                            in0=byteT[:, kt, ts(sl, O_SLAB)],
                            scalar1=SHB - b, scalar2=MASK,
                            op0=mybir.AluOpType.logical_shift_left,
                            op1=mybir.AluOpType.bitwise_and,
                        )
                return wt

            # slab-0 bytes via the PE path; its pk DMA (2.9us) leads the
            # HWDGE queue, then slab-0 unpack leads the DVE queue
            byte_slab(0)
            wt0 = unpack_slab(0)

            # ---- x chunks; kt=0 via HWDGE f32 + ACT cast (first data
            # with no SWDGE descriptor-gen latency), kt>=1 via SWDGE ----
            xns = {}
            for kt in range(KT):
                for tt in range(TT):
                    src_ap = x_d[ts(tt, P), ts(kt, 1024)].rearrange(
                        "p (k b) -> p k b", b=8
                    )
                    xn = xn_pool.tile([P, P, 8], BF16, tag="xn16")
                    if kt == 0:
                        x32 = x32_pool.tile([P, P, 8], F32)
                        nc.sync.dma_start(x32[:], src_ap)
                        nc.scalar.activation(out=xn[:], in_=x32[:], func=ACT_COPY)
                    else:
                        nc.gpsimd.dma_start(xn[:], src_ap)
                    xns[kt, tt] = xn

            # pw16 bounce AFTER the x chunks (an 11us DMA transfer must
            # not block them), split in two so slab-1 bytes arrive first
            nc.gpsimd.dma_start(
                out=pw16_d[O_SLAB:3 * O_SLAB, :], in_=pw_d[O_SLAB:3 * O_SLAB, :]
            )
            for kt in range(KT):
                nc.sync.dma_start_transpose(
                    byteT[:, kt, O_SLAB:3 * O_SLAB],
                    pw16_d[O_SLAB:3 * O_SLAB, ts(kt, P)],
                )
            nc.gpsimd.dma_start(
                out=pw16_d[3 * O_SLAB:, :], in_=pw_d[3 * O_SLAB:, :]
            )
            for kt in range(KT):
                nc.sync.dma_start_transpose(
                    byteT[:, kt, 3 * O_SLAB:],
                    pw16_d[3 * O_SLAB:, ts(kt, P)],
                )

            # ---- transposes, 8 bit-planes of one chunk batched into one
            # [128, 1024] PSUM bank; hi/lo casts read PSUM directly.
            # Each batch depends on a single x chunk.  Slab-1 unpack ops
            # are sprinkled into the DVE stream between the lo casts of
            # the later kt groups so neither pipe starves the other. ----
            wt1 = wt_pool.tile([P, IT, O_SLAB], U16, tag="wt")

            def unpack_part(wt, sl, kts):
                for kt in kts:
                    for b in range(8):
                        it = kt * 8 + b
                        nc.vector.tensor_scalar(
                            out=wt[:, it, :],
                            in0=byteT[:, kt, ts(sl, O_SLAB)],
                            scalar1=SHB - b, scalar2=MASK,
                            op0=mybir.AluOpType.logical_shift_left,
                            op1=mybir.AluOpType.bitwise_and,
                        )

            for kt in range(KT):
                for tt in range(TT):
                    ps = ps_tr.tile([P, 8, P], BF16, tag="tr_ps")
                    for b in range(8):
                        nc.tensor.transpose(
                            ps[:, b, :], xns[kt, tt][:, :, b], ident_bf[:]
                        )
                    hi = xT_hi[:, ts(kt, 8), ts(tt, P)]
                    lo = xT_lo[:, ts(kt, 8), ts(tt, P)]
                    nc.scalar.activation(out=hi, in_=ps[:], func=ACT_COPY)
                    nc.vector.tensor_tensor(
                        out=lo, in0=ps[:], in1=hi, op=mybir.AluOpType.subtract
                    )
                if kt >= 2:
                    unpack_part(wt1, 1, [2 * (kt - 2), 2 * (kt - 2) + 1])

            # ---- main o-slab loop (each group's rank-2 finisher needs
            # s2, which arrives via the s_col DR matmuls below) ----
            def emit_s_col():
                ps_s = ps_tr.tile([P, T], F32, tag="tr_ps")
                for pi, plane in enumerate((xT_hi, xT_lo)):
                    for j in range(NJ):
                        nc.tensor.matmul(
                            ps_s[:], ones2[:], plane[:, 2 * j:2 * j + 2, :],
                            start=(pi == 0 and j == 0),
                            stop=(pi == 1 and j == NJ - 1),
                            perf_mode=DR,
                        )
                nc.vector.tensor_copy(out=s2[0:1, :], in_=ps_s[0:1, :])

            emit_s_col()

            for sl in range(NSLAB):
                if sl == 0:
                    wt = wt0
                elif sl == 1:
                    wt = wt1
                else:
                    wt = unpack_slab(sl)
                wt8 = wt[:].bitcast(F8).rearrange(
                    "p it (n two) -> p it n two", two=2
                )
                for tsub in range(TT):
                    ps = ps_mm.tile([P, O_SLAB], F32)
                    for j in range(NJ):
                        for plane in (xT_hi, xT_lo):
                            nc.tensor.matmul(
                                ps[:],
                                plane[:, 2 * j:2 * j + 2, ts(tsub, P)],
                                wt8[:, 2 * j:2 * j + 2, :, 1],
                                start=(j == 0 and plane is xT_hi),
                                stop=False,
                                perf_mode=DR,
                            )
                    # bias - s_col enters last so the DR stream above can
                    # start before s_col is known
                    nc.tensor.matmul(
                        ps[:], s2[:, ts(tsub, P)], br2[:, ts(sl, O_SLAB)],
                        start=False, stop=True,
                    )
                    y_sb = y_pool.tile([P, O_SLAB], F32)
                    nc.scalar.activation(out=y_sb[:], in_=ps[:], func=ACT_COPY)
                    nc.sync.dma_start(
                        y_d[ts(tsub, P), ts(sl, O_SLAB)], y_sb[:]
                    )

    nc.compile()
    return nc


_NC = None


def _get_nc():
    global _NC
    if _NC is None:
        _NC = build()
    return _NC


def run(x, packed_weight, bias, trace=False):
    x = np.ascontiguousarray(np.asarray(x, dtype=np.float32))
    pw = np.ascontiguousarray(np.asarray(packed_weight, dtype=np.uint8))
    bias = np.ascontiguousarray(np.asarray(bias, dtype=np.float32))
    assert x.shape == (B_DIM, S_DIM, I_DIM)
    assert pw.shape == (O_DIM, I_DIM // 8)
    assert bias.shape == (O_DIM,)

    nc = _get_nc()
    xs = x.reshape(T_FULL, I_DIM)
    in_maps = [
        {
            "x": np.ascontiguousarray(xs[c * T_SHARD:(c + 1) * T_SHARD]),
            "pw": pw,
            "bias": bias,
        }
        for c in range(N_CORES)
    ]
    res = run_bass_kernel_spmd(nc, in_maps, list(range(N_CORES)), trace=trace)
    y = np.concatenate(
        [res.results[c][OUT_NAME] for c in range(N_CORES)], axis=0
    )
    return y.reshape(B_DIM, S_DIM, O_DIM), res


def kernel(x, packed_weight, bias):
    y, _ = run(x, packed_weight, bias, trace=False)
    return y


# revision 34
# speedup vs baseline: 1.5925x; 1.0062x over previous
"""BitPackedLinear Trainium2 kernel (8-core SPMD, token-sharded, fp8 DR).

y = x @ W.T + bias, W = unpack_bits(packed_weight) in {-1,+1}, shapes:
  x [2, 2048, 4096] f32, packed_weight [4096, 512] u8, bias [4096] f32.

Sharding: data-parallel over tokens (4096 tokens -> 512/core). Each core
computes y_c = x_c @ W.T + bias for its token shard against the full
weight; the host just concatenates shards.

Device algorithm per core (main GEMM in fp8e4 DoubleRow, 0.5 cyc/row --
2x the bf16 row rate, contracting two 128-i blocks per instruction):
  - x is split exactly as x = hi + lo with hi = fp8(bf16(x)),
    lo = fp8(bf16(x) - hi); W is unpacked to {0, 2.0} and the result
    corrected as y = (hi+lo)@(2B) - rowsum(hi+lo) + bias.  Total split
    error ~1.2e-3 rel (bf16-level).
  - x chunks arrive bf16 (SWDGE cast-DMA; kt=0 via HWDGE f32 + DVE cast
    to dodge the slow Q7 descriptor-gen start), bit-sliced layout.
  - PE transposes x per [128,128] bf16 tile into per-kt groups, then one
    ACT copy casts the group to the fp8 hi plane and one DVE subtract
    produces the fp8 lo plane: xT_hi/xT_lo [128 i-part, 32 it, 512 tok],
    tokens contiguous (the dual-fp8 Ldweights ISA check requires a
    unit-stride stationary m dim; the moving ifmap tolerates stride 2).
  - byteT[k',kt,o] = pw[o,128kt+k'] via u8->u16 cast-DMA bounce through
    DRAM + one XBAR transpose-DMA per kt (no PE/DVE involvement).
  - W unpack stays 1 op/tile: wt16 = (u16(byte) << (14-b)) & 0x4000
    puts fp8 {0, 2.0} in the HIGH byte; the DR matmul reads odd bytes
    ([p, 2, 512] pair-stride 1024, n-stride 2 -- validated on hw).
  - Main DR matmuls per (o-slab, token-tile), for j in 0..16, term in
    (hi, lo): psum += plane[:, 2j:2j+2, t].T (x) wt8[:, 2j:2j+2, :].
  - s_col = rowsum(hi+lo) computed EXACTLY by 32 DR matmuls against an
    all-ones stationary -> psum row -> SBUF f32r row.
  - bias + (-s_col) enter each psum via ONE rank-2 f32r matmul
    (k=0: s_col x -1, k=1: ones x bias) with start=True; epilogue is a
    plain psum->SBUF copy + DMA.
"""
import sys

sys.path.insert(0, "/opt/trn_rl_repo")
from contextlib import ExitStack

import numpy as np

import concourse.tile as tile
from concourse import bacc, mybir
from concourse.bass import ts
from concourse.bass_utils import run_bass_kernel_spmd
from concourse.masks import make_identity

F32 = mybir.dt.float32
F32R = mybir.dt.float32r
BF16 = mybir.dt.bfloat16
F8 = mybir.dt.float8e4
U8 = mybir.dt.uint8
U16 = mybir.dt.uint16
P = 128
DR = mybir.MatmulPerfMode.DoubleRow
ACT_COPY = mybir.ActivationFunctionType.Copy

N_CORES = 8
B_DIM, S_DIM, I_DIM, O_DIM = 2, 2048, 4096, 4096
T_FULL = B_DIM * S_DIM          # 4096 tokens
T_SHARD = T_FULL // N_CORES     # 512 tokens per core
OUT_NAME = "y"
OUT_SHAPE = (T_SHARD, O_DIM)


def build(T=T_SHARD, I=I_DIM, O=O_DIM, O_SLAB=512, n_cores=N_CORES,
          byte_mode="dmat"):
    assert I % 1024 == 0 and T % P == 0 and O % P == 0 and O % O_SLAB == 0
    KT = I // 1024          # 128-byte groups along i (4)
    IT = KT * 8             # bit-sliced i-tiles (32)
    NJ = IT // 2            # DR i-tile pairs (16)
    TT = T // P             # token tiles (4)
    K = I // 8              # packed bytes per weight row (512)
    NSLAB = O // O_SLAB
    OSL_T = O_SLAB // P
    SHB, MASK = 14, 0x4000  # unpack: fp8 {0,2.0} pattern in the HIGH byte

    nc = bacc.Bacc("TRN2", target_bir_lowering=False, debug=False,
                   num_devices=n_cores)
    x_d = nc.dram_tensor("x", [T, I], F32, kind="ExternalInput").ap()
    pw_d = nc.dram_tensor("pw", [O, K], U8, kind="ExternalInput").ap()
    bias_d = nc.dram_tensor("bias", [O], F32, kind="ExternalInput").ap()
    y_d = nc.dram_tensor(OUT_NAME, [T, O], F32, kind="ExternalOutput").ap()
    pw16_d = nc.dram_tensor("pw16", [O, K], U16).ap()

    with tile.TileContext(nc) as tc:
        with ExitStack() as ctx:
            const = ctx.enter_context(tc.tile_pool(name="const", bufs=1))
            persist = ctx.enter_context(tc.tile_pool(name="persist", bufs=1))
            stage = ctx.enter_context(tc.tile_pool(name="stage", bufs=1))

            ident_bf = const.tile([P, P], BF16)
            make_identity(nc, ident_bf[:])
            ones2 = const.tile([P, 2, P], F8)
            nc.vector.memset(ones2[:], 1.0)

            # rank-2 psum-init operands; engine-written rows sit at
            # partition 0 (engine APs must start on a quarter boundary),
            # partition-1 rows are DMA-filled.
            # ps = s2[0,t]*br2[0,o] + s2[1,t]*br2[1,o]
            #    = s_col[t]*(-1)    + 1*bias[o]
            s2 = const.tile([2, T], F32R)    # row0 = s_col, row1 = ones
            br2 = const.tile([2, O], F32R)   # row0 = -ones, row1 = bias

            byteT = persist.tile([P, KT, O], U16)
            xT_hi = persist.tile([P, IT, T], F8)
            xT_lo = persist.tile([P, IT, T], F8)

            x32_pool = ctx.enter_context(tc.tile_pool(name="x32", bufs=2))
            xn_pool = ctx.enter_context(tc.tile_pool(name="xnat", bufs=10))
            wt_pool = ctx.enter_context(tc.tile_pool(name="wt", bufs=2))
            y_pool = ctx.enter_context(tc.tile_pool(name="ysb", bufs=3))
            ps_tr = ctx.enter_context(
                tc.tile_pool(name="ps_tr", bufs=3, space="PSUM")
            )
            ps_mm = ctx.enter_context(
                tc.tile_pool(name="ps_mm", bufs=5, space="PSUM")
            )


            def emit_bias_stage():
                nc.sync.dma_start(
                    br2[1:2, :],
                    bias_d.bitcast(F32R).rearrange("(b o) -> b o", b=1),
                )
                cst = stage.tile([1, T], F32)
                nc.vector.memset(cst[:], -1.0)
                nc.vector.tensor_copy(
                    out=br2[0:1, :],
                    in_=cst[:, :1].broadcast_to([1, O]),
                )
                ones_row = stage.tile([1, T], F32R)
                nc.vector.memset(cst[:], 1.0)
                nc.vector.tensor_copy(out=ones_row[:], in_=cst[:])
                nc.sync.dma_start(s2[1:2, :], ones_row[:])

            emit_bias_stage()

            # PE warm-up: back-to-back identity transposes ramp the
            # p-state to full clock while the first DMAs land.
            ps_warm = ps_tr.tile([P, 8, P], BF16, tag="tr_ps")
            for i in range(24):
                nc.tensor.transpose(
                    ps_warm[:, i % 8, :], ident_bf[:], ident_bf[:]
                )

            # slab-0/1 unpacks lead the DVE queue
            def unpack_slab(sl):
                wt = wt_pool.tile([P, IT, O_SLAB], U16, tag="wt")
                for kt in range(KT):
                    for b in range(8):
                        it = kt * 8 + b
                        nc.vector.tensor_scalar(
                            out=wt[:, it, :],
                            in0=byteT[:, kt, ts(sl, O_SLAB)],
                            scalar1=SHB - b, scalar2=MASK,
                            op0=mybir.AluOpType.logical_shift_left,
                            op1=mybir.AluOpType.bitwise_and,
                        )
                return wt

            # ---- byte path: pw16 bounce split so slabs 0-1 XBAR in
            # first (slab-0 unpack then leads the DVE queue) ----
            nc.gpsimd.dma_start(
                out=pw16_d[:2 * O_SLAB, :], in_=pw_d[:2 * O_SLAB, :]
            )
            for kt in range(KT):
                nc.sync.dma_start_transpose(
                    byteT[:, kt, :2 * O_SLAB],
                    pw16_d[:2 * O_SLAB, ts(kt, P)],
                )
            wt0 = unpack_slab(0)

            # ---- x chunks, tt-major so token-tile 0's four kt chunks
            # arrive first and group (sl0, tt0) unblocks earliest.
            # kt=0 via HWDGE f32 + ACT cast, kt>=1 via SWDGE cast-DMA ----
            xns = {}
            for tt in range(TT):
                for kt in range(KT):
                    src_ap = x_d[ts(tt, P), ts(kt, 1024)].rearrange(
                        "p (k b) -> p k b", b=8
                    )
                    xn = xn_pool.tile([P, P, 8], BF16, tag="xn16")
                    if kt == 0:
                        x32 = x32_pool.tile([P, P, 8], F32)
                        nc.sync.dma_start(x32[:], src_ap)
                        nc.scalar.activation(out=xn[:], in_=x32[:], func=ACT_COPY)
                    else:
                        nc.gpsimd.dma_start(xn[:], src_ap)
                    xns[kt, tt] = xn

            # rest of the pw16 bounce + XBARs (slabs 2-7)
            nc.gpsimd.dma_start(
                out=pw16_d[2 * O_SLAB:, :], in_=pw_d[2 * O_SLAB:, :]
            )
            for kt in range(KT):
                nc.sync.dma_start_transpose(
                    byteT[:, kt, 2 * O_SLAB:],
                    pw16_d[2 * O_SLAB:, ts(kt, P)],
                )

            # ---- transposes tt-major, 8 bit-planes of one chunk batched
            # into one [128, 1024] PSUM bank; hi (ACT) / lo (DVE) casts
            # read PSUM directly ----
            for tt in range(TT):
                for kt in range(KT):
                    ps = ps_tr.tile([P, 8, P], BF16, tag="tr_ps")
                    for b in range(8):
                        nc.tensor.transpose(
                            ps[:, b, :], xns[kt, tt][:, :, b], ident_bf[:]
                        )
                    hi = xT_hi[:, ts(kt, 8), ts(tt, P)]
                    lo = xT_lo[:, ts(kt, 8), ts(tt, P)]
                    nc.scalar.activation(out=hi, in_=ps[:], func=ACT_COPY)
                    nc.vector.tensor_tensor(
                        out=lo, in0=ps[:], in1=hi, op=mybir.AluOpType.subtract
                    )

            # ---- main o-slab loop.  Slab 0 is special: its 4 groups'
            # DR streams are emitted first (they only need wt0 and their
            # own token-tile's planes), then the s_col DR matmuls (which
            # need ALL planes), then the rank-2 finishers -- so the
            # in-order PE queue never parks on s2. ----
            def emit_group_drs(ps, wt8, tsub):
                for j in range(NJ):
                    for plane in (xT_hi, xT_lo):
                        nc.tensor.matmul(
                            ps[:],
                            plane[:, 2 * j:2 * j + 2, ts(tsub, P)],
                            wt8[:, 2 * j:2 * j + 2, :, 1],
                            start=(j == 0 and plane is xT_hi),
                            stop=False,
                            perf_mode=DR,
                        )

            def emit_finish(ps, sl, tsub):
                # bias - s_col enters last; closes the accumulation group
                nc.tensor.matmul(
                    ps[:], s2[:, ts(tsub, P)], br2[:, ts(sl, O_SLAB)],
                    start=False, stop=True,
                )
                y_sb = y_pool.tile([P, O_SLAB], F32)
                nc.scalar.activation(out=y_sb[:], in_=ps[:], func=ACT_COPY)
                nc.sync.dma_start(
                    y_d[ts(tsub, P), ts(sl, O_SLAB)], y_sb[:]
                )

            wt0_8 = wt0[:].bitcast(F8).rearrange(
                "p it (n two) -> p it n two", two=2
            )
            ps0 = []
            for tsub in range(TT):
                ps = ps_mm.tile([P, O_SLAB], F32)
                emit_group_drs(ps, wt0_8, tsub)
                ps0.append(ps)

            # s_col = rowsum(hi+lo) via DR matmuls vs an all-ones
            # stationary; then the s2 row and slab-0 finishers
            ps_s = ps_tr.tile([P, T], F32, tag="tr_ps")
            for pi, plane in enumerate((xT_hi, xT_lo)):
                for j in range(NJ):
                    nc.tensor.matmul(
                        ps_s[:], ones2[:], plane[:, 2 * j:2 * j + 2, :],
                        start=(pi == 0 and j == 0),
                        stop=(pi == 1 and j == NJ - 1),
                        perf_mode=DR,
                    )
            nc.vector.tensor_copy(out=s2[0:1, :], in_=ps_s[0:1, :])
            for tsub in range(TT):
                emit_finish(ps0[tsub], 0, tsub)

            wt1 = unpack_slab(1)

            for sl in range(1, NSLAB):
                wt = wt1 if sl == 1 else unpack_slab(sl)
                wt8 = wt[:].bitcast(F8).rearrange(
                    "p it (n two) -> p it n two", two=2
                )
                for tsub in range(TT):
                    ps = ps_mm.tile([P, O_SLAB], F32)
                    emit_group_drs(ps, wt8, tsub)
                    emit_finish(ps, sl, tsub)

    nc.compile()
    return nc


_NC = None


def _get_nc():
    global _NC
    if _NC is None:
        _NC = build()
    return _NC


def run(x, packed_weight, bias, trace=False):
    x = np.ascontiguousarray(np.asarray(x, dtype=np.float32))
    pw = np.ascontiguousarray(np.asarray(packed_weight, dtype=np.uint8))
    bias = np.ascontiguousarray(np.asarray(bias, dtype=np.float32))
    assert x.shape == (B_DIM, S_DIM, I_DIM)
    assert pw.shape == (O_DIM, I_DIM // 8)
    assert bias.shape == (O_DIM,)

    nc = _get_nc()
    xs = x.reshape(T_FULL, I_DIM)
    in_maps = [
        {
            "x": np.ascontiguousarray(xs[c * T_SHARD:(c + 1) * T_SHARD]),
            "pw": pw,
            "bias": bias,
        }
        for c in range(N_CORES)
    ]
    res = run_bass_kernel_spmd(nc, in_maps, list(range(N_CORES)), trace=trace)
    y = np.concatenate(
        [res.results[c][OUT_NAME] for c in range(N_CORES)], axis=0
    )
    return y.reshape(B_DIM, S_DIM, O_DIM), res


def kernel(x, packed_weight, bias):
    y, _ = run(x, packed_weight, bias, trace=False)
    return y


# revision 35
# speedup vs baseline: 1.6405x; 1.0301x over previous
"""BitPackedLinear Trainium2 kernel (8-core SPMD, token-sharded, fp8 DR).

y = x @ W.T + bias, W = unpack_bits(packed_weight) in {-1,+1}, shapes:
  x [2, 2048, 4096] f32, packed_weight [4096, 512] u8, bias [4096] f32.

Sharding: data-parallel over tokens (4096 tokens -> 512/core). Each core
computes y_c = x_c @ W.T + bias for its token shard against the full
weight; the host just concatenates shards.

Device algorithm per core (main GEMM in fp8e4 DoubleRow, 0.5 cyc/row --
2x the bf16 row rate, contracting two 128-i blocks per instruction):
  - x is split exactly as x = hi + lo with hi = fp8(bf16(x)),
    lo = fp8(bf16(x) - hi); W is unpacked to {0, 2.0} and the result
    corrected as y = (hi+lo)@(2B) - rowsum(hi+lo) + bias.  Total split
    error ~1.2e-3 rel (bf16-level).
  - x chunks arrive bf16 (SWDGE cast-DMA; kt=0 via HWDGE f32 + DVE cast
    to dodge the slow Q7 descriptor-gen start), bit-sliced layout.
  - PE transposes x per [128,128] bf16 tile into per-kt groups, then one
    ACT copy casts the group to the fp8 hi plane and one DVE subtract
    produces the fp8 lo plane: xT_hi/xT_lo [128 i-part, 32 it, 512 tok],
    tokens contiguous (the dual-fp8 Ldweights ISA check requires a
    unit-stride stationary m dim; the moving ifmap tolerates stride 2).
  - byteT[k',kt,o] = pw[o,128kt+k'] via u8->u16 cast-DMA bounce through
    DRAM + one XBAR transpose-DMA per kt (no PE/DVE involvement).
  - W unpack stays 1 op/tile: wt16 = (u16(byte) << (14-b)) & 0x4000
    puts fp8 {0, 2.0} in the HIGH byte; the DR matmul reads odd bytes
    ([p, 2, 512] pair-stride 1024, n-stride 2 -- validated on hw).
  - Main DR matmuls per (o-slab, token-tile), for j in 0..16, term in
    (hi, lo): psum += plane[:, 2j:2j+2, t].T (x) wt8[:, 2j:2j+2, :].
  - s_col = rowsum(hi+lo) computed EXACTLY by 32 DR matmuls against an
    all-ones stationary -> psum row -> SBUF f32r row.
  - bias + (-s_col) enter each psum via ONE rank-2 f32r matmul
    (k=0: s_col x -1, k=1: ones x bias) with start=True; epilogue is a
    plain psum->SBUF copy + DMA.
"""
import sys

sys.path.insert(0, "/opt/trn_rl_repo")
from contextlib import ExitStack

import numpy as np

import concourse.tile as tile
from concourse import bacc, mybir
from concourse.bass import ts
from concourse.bass_utils import run_bass_kernel_spmd
from concourse.masks import make_identity

F32 = mybir.dt.float32
F32R = mybir.dt.float32r
BF16 = mybir.dt.bfloat16
F8 = mybir.dt.float8e4
U8 = mybir.dt.uint8
U16 = mybir.dt.uint16
P = 128
DR = mybir.MatmulPerfMode.DoubleRow
ACT_COPY = mybir.ActivationFunctionType.Copy

N_CORES = 8
B_DIM, S_DIM, I_DIM, O_DIM = 2, 2048, 4096, 4096
T_FULL = B_DIM * S_DIM          # 4096 tokens
T_SHARD = T_FULL // N_CORES     # 512 tokens per core
OUT_NAME = "y"
OUT_SHAPE = (T_SHARD, O_DIM)


def build(T=T_SHARD, I=I_DIM, O=O_DIM, O_SLAB=512, n_cores=N_CORES,
          byte_mode="dmat"):
    assert I % 1024 == 0 and T % P == 0 and O % P == 0 and O % O_SLAB == 0
    KT = I // 1024          # 128-byte groups along i (4)
    IT = KT * 8             # bit-sliced i-tiles (32)
    NJ = IT // 2            # DR i-tile pairs (16)
    TT = T // P             # token tiles (4)
    K = I // 8              # packed bytes per weight row (512)
    NSLAB = O // O_SLAB
    OSL_T = O_SLAB // P
    SHB, MASK = 14, 0x4000  # unpack: fp8 {0,2.0} pattern in the HIGH byte

    nc = bacc.Bacc("TRN2", target_bir_lowering=False, debug=False,
                   num_devices=n_cores)
    x_d = nc.dram_tensor("x", [T, I], F32, kind="ExternalInput").ap()
    pw_d = nc.dram_tensor("pw", [O, K], U8, kind="ExternalInput").ap()
    bias_d = nc.dram_tensor("bias", [O], F32, kind="ExternalInput").ap()
    y_d = nc.dram_tensor(OUT_NAME, [T, O], F32, kind="ExternalOutput").ap()
    pw16_d = nc.dram_tensor("pw16", [O, K], U16).ap()

    with tile.TileContext(nc) as tc:
        with ExitStack() as ctx:
            const = ctx.enter_context(tc.tile_pool(name="const", bufs=1))
            persist = ctx.enter_context(tc.tile_pool(name="persist", bufs=1))
            stage = ctx.enter_context(tc.tile_pool(name="stage", bufs=1))

            ident_bf = const.tile([P, P], BF16)
            make_identity(nc, ident_bf[:])
            ones2 = const.tile([P, 2, P], F8)
            nc.vector.memset(ones2[:], 1.0)

            # rank-2 psum-init operands; engine-written rows sit at
            # partition 0 (engine APs must start on a quarter boundary),
            # partition-1 rows are DMA-filled.
            # ps = s2[0,t]*br2[0,o] + s2[1,t]*br2[1,o]
            #    = s_col[t]*(-1)    + 1*bias[o]
            s2 = const.tile([2, T], F32R)    # row0 = s_col, row1 = ones
            br2 = const.tile([2, O], F32R)   # row0 = -ones, row1 = bias

            byteT = persist.tile([P, KT, O], U16)
            xT_hi = persist.tile([P, IT, T], F8)
            xT_lo = persist.tile([P, IT, T], F8)

            x32_pool = ctx.enter_context(tc.tile_pool(name="x32", bufs=2))
            xn_pool = ctx.enter_context(tc.tile_pool(name="xnat", bufs=10))
            pk_pool = ctx.enter_context(tc.tile_pool(name="pk", bufs=2))
            pkbf_pool = ctx.enter_context(tc.tile_pool(name="pkbf", bufs=2))
            wt_pool = ctx.enter_context(tc.tile_pool(name="wt", bufs=2))
            y_pool = ctx.enter_context(tc.tile_pool(name="ysb", bufs=3))
            ps_tr = ctx.enter_context(
                tc.tile_pool(name="ps_tr", bufs=3, space="PSUM")
            )
            ps_mm = ctx.enter_context(
                tc.tile_pool(name="ps_mm", bufs=5, space="PSUM")
            )


            def emit_bias_stage():
                nc.sync.dma_start(
                    br2[1:2, :],
                    bias_d.bitcast(F32R).rearrange("(b o) -> b o", b=1),
                )
                cst = stage.tile([1, T], F32)
                nc.vector.memset(cst[:], -1.0)
                nc.vector.tensor_copy(
                    out=br2[0:1, :],
                    in_=cst[:, :1].broadcast_to([1, O]),
                )
                ones_row = stage.tile([1, T], F32R)
                nc.vector.memset(cst[:], 1.0)
                nc.vector.tensor_copy(out=ones_row[:], in_=cst[:])
                return ones_row

            ones_row = emit_bias_stage()

            def byte_slab(sl):
                """Fill byteT[:, :, sl*O_SLAB:(sl+1)*O_SLAB] via the PE
                (latency-free vs the XBAR queue for early slabs)."""
                pw_ap = pw_d.rearrange("(ot p) k -> p ot k", p=P)
                pk = pk_pool.tile([P, OSL_T, K], U8)
                nc.sync.dma_start(pk[:], pw_ap[:, ts(sl, OSL_T), :])
                for otl in range(OSL_T):
                    ot = sl * OSL_T + otl
                    pkbf = pkbf_pool.tile([P, K], BF16)
                    nc.scalar.activation(out=pkbf[:], in_=pk[:, otl, :],
                                         func=ACT_COPY)
                    ps = ps_tr.tile([P, 8, P], BF16, tag="tr_ps")
                    for kt in range(KT):
                        nc.tensor.transpose(
                            ps[:, kt, :], pkbf[:, ts(kt, P)], ident_bf[:]
                        )
                    dst = byteT[:, :, ts(ot, P)]
                    if otl % 2 == 0:
                        nc.vector.tensor_copy(out=dst, in_=ps[:, :KT, :])
                    else:
                        nc.scalar.activation(out=dst, in_=ps[:, :KT, :],
                                             func=ACT_COPY)

            # PE warm-up: back-to-back identity transposes ramp the
            # p-state to full clock while the first DMAs land.
            ps_warm = ps_tr.tile([P, 8, P], BF16, tag="tr_ps")
            for i in range(24):
                nc.tensor.transpose(
                    ps_warm[:, i % 8, :], ident_bf[:], ident_bf[:]
                )

            # slab-0/1 unpacks lead the DVE queue
            def unpack_slab(sl):
                wt = wt_pool.tile([P, IT, O_SLAB], U16, tag="wt")
                for kt in range(KT):
                    for b in range(8):
                        it = kt * 8 + b
                        nc.vector.tensor_scalar(
                            out=wt[:, it, :],
                            in0=byteT[:, kt, ts(sl, O_SLAB)],
                            scalar1=SHB - b, scalar2=MASK,
                            op0=mybir.AluOpType.logical_shift_left,
                            op1=mybir.AluOpType.bitwise_and,
                        )
                return wt

            # ---- byte path: slabs 0-1 via the PE (the XBAR/SWDGE
            # bounce has ~6us per-DMA queue latency; early slabs cannot
            # wait for it) ----
            byte_slab(0)
            wt0 = unpack_slab(0)
            byte_slab(1)

            # ---- x chunks, tt-major so token-tile 0's four kt chunks
            # arrive first and group (sl0, tt0) unblocks earliest.
            # kt=0 via HWDGE f32 + ACT cast, kt>=1 via SWDGE cast-DMA ----
            xns = {}
            for tt in range(TT):
                for kt in range(KT):
                    src_ap = x_d[ts(tt, P), ts(kt, 1024)].rearrange(
                        "p (k b) -> p k b", b=8
                    )
                    xn = xn_pool.tile([P, P, 8], BF16, tag="xn16")
                    if kt == 0:
                        x32 = x32_pool.tile([P, P, 8], F32)
                        nc.sync.dma_start(x32[:], src_ap)
                        nc.scalar.activation(out=xn[:], in_=x32[:], func=ACT_COPY)
                    else:
                        nc.gpsimd.dma_start(xn[:], src_ap)
                    xns[kt, tt] = xn

            # s2 ones row (DMA here so it doesn't head-block the sync
            # queue while waiting for the DVE-built constant)
            nc.sync.dma_start(s2[1:2, :], ones_row[:])

            # pw16 bounce + XBARs for slabs 2-7 (plenty of slack: slab 2
            # isn't consumed until ~50us in)
            nc.gpsimd.dma_start(
                out=pw16_d[2 * O_SLAB:, :], in_=pw_d[2 * O_SLAB:, :]
            )
            for kt in range(KT):
                nc.sync.dma_start_transpose(
                    byteT[:, kt, 2 * O_SLAB:],
                    pw16_d[2 * O_SLAB:, ts(kt, P)],
                )

            # ---- transposes tt-major, 8 bit-planes of one chunk batched
            # into one [128, 1024] PSUM bank; hi (ACT) / lo (DVE) casts
            # read PSUM directly ----
            for tt in range(TT):
                for kt in range(KT):
                    ps = ps_tr.tile([P, 8, P], BF16, tag="tr_ps")
                    for b in range(8):
                        nc.tensor.transpose(
                            ps[:, b, :], xns[kt, tt][:, :, b], ident_bf[:]
                        )
                    hi = xT_hi[:, ts(kt, 8), ts(tt, P)]
                    lo = xT_lo[:, ts(kt, 8), ts(tt, P)]
                    nc.scalar.activation(out=hi, in_=ps[:], func=ACT_COPY)
                    nc.vector.tensor_tensor(
                        out=lo, in0=ps[:], in1=hi, op=mybir.AluOpType.subtract
                    )

            # ---- main o-slab loop.  Slab 0 is special: its 4 groups'
            # DR streams are emitted first (they only need wt0 and their
            # own token-tile's planes), then the s_col DR matmuls (which
            # need ALL planes), then the rank-2 finishers -- so the
            # in-order PE queue never parks on s2. ----
            def emit_group_drs(ps, wt8, tsub):
                for j in range(NJ):
                    for plane in (xT_hi, xT_lo):
                        nc.tensor.matmul(
                            ps[:],
                            plane[:, 2 * j:2 * j + 2, ts(tsub, P)],
                            wt8[:, 2 * j:2 * j + 2, :, 1],
                            start=(j == 0 and plane is xT_hi),
                            stop=False,
                            perf_mode=DR,
                        )

            def emit_finish(ps, sl, tsub):
                # bias - s_col enters last; closes the accumulation group
                nc.tensor.matmul(
                    ps[:], s2[:, ts(tsub, P)], br2[:, ts(sl, O_SLAB)],
                    start=False, stop=True,
                )
                y_sb = y_pool.tile([P, O_SLAB], F32)
                nc.scalar.activation(out=y_sb[:], in_=ps[:], func=ACT_COPY)
                nc.sync.dma_start(
                    y_d[ts(tsub, P), ts(sl, O_SLAB)], y_sb[:]
                )

            wt0_8 = wt0[:].bitcast(F8).rearrange(
                "p it (n two) -> p it n two", two=2
            )
            ps0 = []
            for tsub in range(TT):
                ps = ps_mm.tile([P, O_SLAB], F32)
                emit_group_drs(ps, wt0_8, tsub)
                ps0.append(ps)

            # s_col = rowsum(hi+lo) via DR matmuls vs an all-ones
            # stationary; then the s2 row and slab-0 finishers
            ps_s = ps_tr.tile([P, T], F32, tag="tr_ps")
            for pi, plane in enumerate((xT_hi, xT_lo)):
                for j in range(NJ):
                    nc.tensor.matmul(
                        ps_s[:], ones2[:], plane[:, 2 * j:2 * j + 2, :],
                        start=(pi == 0 and j == 0),
                        stop=(pi == 1 and j == NJ - 1),
                        perf_mode=DR,
                    )
            nc.vector.tensor_copy(out=s2[0:1, :], in_=ps_s[0:1, :])
            for tsub in range(TT):
                emit_finish(ps0[tsub], 0, tsub)

            wt1 = unpack_slab(1)

            for sl in range(1, NSLAB):
                wt = wt1 if sl == 1 else unpack_slab(sl)
                wt8 = wt[:].bitcast(F8).rearrange(
                    "p it (n two) -> p it n two", two=2
                )
                for tsub in range(TT):
                    ps = ps_mm.tile([P, O_SLAB], F32)
                    emit_group_drs(ps, wt8, tsub)
                    emit_finish(ps, sl, tsub)

    nc.compile()
    return nc


_NC = None


def _get_nc():
    global _NC
    if _NC is None:
        _NC = build()
    return _NC


def run(x, packed_weight, bias, trace=False):
    x = np.ascontiguousarray(np.asarray(x, dtype=np.float32))
    pw = np.ascontiguousarray(np.asarray(packed_weight, dtype=np.uint8))
    bias = np.ascontiguousarray(np.asarray(bias, dtype=np.float32))
    assert x.shape == (B_DIM, S_DIM, I_DIM)
    assert pw.shape == (O_DIM, I_DIM // 8)
    assert bias.shape == (O_DIM,)

    nc = _get_nc()
    xs = x.reshape(T_FULL, I_DIM)
    in_maps = [
        {
            "x": np.ascontiguousarray(xs[c * T_SHARD:(c + 1) * T_SHARD]),
            "pw": pw,
            "bias": bias,
        }
        for c in range(N_CORES)
    ]
    res = run_bass_kernel_spmd(nc, in_maps, list(range(N_CORES)), trace=trace)
    y = np.concatenate(
        [res.results[c][OUT_NAME] for c in range(N_CORES)], axis=0
    )
    return y.reshape(B_DIM, S_DIM, O_DIM), res


def kernel(x, packed_weight, bias):
    y, _ = run(x, packed_weight, bias, trace=False)
    return y
